# revision 1
# baseline (speedup 1.0000x reference)
"""Trainium2 Bass kernel for nn_CABlock (channel attention / XCA block).

Reference computation (per batch b):
  qkv = x @ qkv_w.T                      # [N, 3C], token-major
  q,k,v per head: [d=64, N] channel-major after reshape/transpose
  q,k l2-normalized over N; attn = softmax((q @ k.T) * temperature, axis=-1)
  out = attn @ v  -> [N, C];  y = out @ proj_w.T + proj_b

Key algebraic restructure: l2norm commutes with the bilinear form, so
  attn_logits = diag(inv_q) @ (q_raw @ k_raw.T) @ diag(inv_k) * temp
with inv_q[c] = 1/max(||q[c,:]||,eps). We accumulate q^T k Gram blocks and
per-channel sums of squares (via a ones-vector matmul over squared values)
in PSUM across all token chunks, then apply the tiny [64x64]-per-head
softmax at the end. This avoids ever materializing normalized q/k.

Sharding: data-parallel over batch B=16 across 8 cores (2 batches/core).
No collectives needed.
"""

import os
import sys

import numpy as np

for _p in ("/opt/trn_rl_repo", "/root/.axon_site/_ro/trn_rl_repo"):
    if os.path.isdir(_p) and _p not in sys.path:
        sys.path.insert(0, _p)

import concourse.bass as bass  # noqa: E402
from concourse import mybir  # noqa: E402
from concourse.bass import ts  # noqa: E402
from concourse.bass_utils import run_bass_kernel_spmd  # noqa: E402
from concourse.masks import make_identity  # noqa: E402
from concourse.tile import TileContext  # noqa: E402

B, N, C = 16, 4096, 512
H, D = 8, 64
C3 = 3 * C
NCORES = 8
BL = B // NCORES  # batches per core
EPS = 1e-12
NCHUNK = N // 128  # 32 token chunks per batch
F32 = mybir.dt.float32

# Matmul operand mode: "bf16" | "fp32" | "f32r" (set BASS_MM_MODE to override)
MM_MODE = os.environ.get("BASS_MM_MODE", "bf16")


def legalize_waits(nc):
    """Walrus in this environment rejects instructions carrying more than one
    semaphore wait ("Too many sync wait commands"), and rejects sem-ge waits
    on Drain instructions entirely. Tile emits both. Hoist the offending
    waits onto standalone EventSemaphore instructions inserted immediately
    before the instruction on the same engine queue — semantically identical
    (the engine executes the waits, then the instruction)."""
    n_new = 0
    for bb in nc.main_func.blocks:
        il = bb.instructions
        new_list = []
        for ins in il:
            si = ins.sync_info
            waits = list(si.on_wait) if si is not None and si.on_wait else []
            if waits:
                tname = type(ins).__name__
                no_wait_slots = tname in ("InstDrain", "InstDmaTransposeAnt") or (
                    getattr(ins, "opcode", "") in ("Drain", "DmaTransposeAnt")
                )
                keep_budget = 0 if no_wait_slots else 1
                if len(waits) > keep_budget:
                    hoist, keep = waits[:-keep_budget] if keep_budget else waits, (
                        waits[-keep_budget:] if keep_budget else []
                    )
                    for w in hoist:
                        ev = mybir.InstEventSemaphore(
                            name=f"{ins.name}-hoistw{n_new}",
                            ins=[],
                            outs=[],
                            engine=ins.engine,
                            sync_info=mybir.SyncInfo(on_wait=[w], on_update=[]),
                        )
                        new_list.append(ev)
                        n_new += 1
                    ins.sync_info = mybir.SyncInfo(
                        on_wait=keep, on_update=list(si.on_update or [])
                    )
            new_list.append(ins)
        il.clear()
        il.extend(new_list)
    return n_new


def build_bass():
    mode = MM_MODE
    op_dt = mybir.dt.bfloat16 if mode == "bf16" else F32

    def mm(ap):
        """Cast an operand AP at a matmul call site for the big matmuls."""
        if mode == "f32r":
            return ap.bitcast(mybir.dt.float32r)
        return ap

    nc = bass.Bass(trn_type="TRN2")
    x = nc.dram_tensor("x", [BL, N, C], F32, kind="ExternalInput")
    # weights arrive pre-transposed ([cin, cout]) and pre-converted to the
    # matmul dtype by the host wrapper
    qkv_wt = nc.dram_tensor("qkv_wt", [C, C3], op_dt, kind="ExternalInput")
    temp = nc.dram_tensor("temperature", [H], F32, kind="ExternalInput")
    proj_wt = nc.dram_tensor("proj_wt", [C, C], op_dt, kind="ExternalInput")
    proj_b = nc.dram_tensor("proj_b", [C], F32, kind="ExternalInput")
    out = nc.dram_tensor("out", [BL, N, C], F32, kind="ExternalOutput")

    with TileContext(nc) as tc:
        consts = tc.alloc_tile_pool(name="consts", bufs=1)
        xin = tc.alloc_tile_pool(name="xin", bufs=6)
        chunk = tc.alloc_tile_pool(name="chunk", bufs=3)
        # fp32 modes double the vT footprint; drop cross-batch double-buffering
        vtp = tc.alloc_tile_pool(name="vtp", bufs=8 if op_dt != F32 else 4)
        small = tc.alloc_tile_pool(name="small", bufs=2)
        outp = tc.alloc_tile_pool(name="outp", bufs=2)
        yp = tc.alloc_tile_pool(name="yp", bufs=3)
        ps = tc.alloc_tile_pool(name="ps", bufs=5, space="PSUM")
        accp = tc.alloc_tile_pool(name="accp", bufs=1, space="PSUM")

        # ---- constants ----
        ident = consts.tile([128, 128], op_dt)
        make_identity(nc, ident)
        ones_col = consts.tile([128, 1], op_dt)
        nc.vector.memset(ones_col, 1.0)
        ones_f32 = consts.tile([1, 128], F32)
        nc.vector.memset(ones_f32, 1.0)
        id1_f32 = consts.tile([1, 1], F32)
        nc.vector.memset(id1_f32, 1.0)

        # temperature: [1, H] row (broadcast along D later via 0-step APs)
        temp_sb = consts.tile([1, H], F32)
        nc.sync.dma_start(out=temp_sb, in_=temp[:])

        # proj bias: load [1, C] then broadcast to all partitions via a
        # K=1 ones-matmul on the PE (out[p, c] = 1 * bias[c])
        bias_row = consts.tile([1, C], F32)
        nc.sync.dma_start(out=bias_row, in_=proj_b[:])
        bias_ps = ps.tile([128, C], F32, tag="ps")
        nc.tensor.matmul(bias_ps, ones_f32, bias_row, start=True, stop=True)
        bias_bc = consts.tile([128, C], F32)
        nc.vector.tensor_copy(out=bias_bc, in_=bias_ps)

        # ---- weights: already [cin, cout] in matmul dtype; plain DMA loads ----
        qkv_wT = [consts.tile([128, C3], op_dt, tag=f"qkvwT{i}", name=f"qkvwT{i}") for i in range(4)]
        proj_wT = [consts.tile([128, C], op_dt, tag=f"projwT{i}", name=f"projwT{i}") for i in range(4)]
        for kc in range(4):
            nc.sync.dma_start(out=qkv_wT[kc], in_=qkv_wt[ts(kc, 128), :])
            nc.sync.dma_start(out=proj_wT[kc], in_=proj_wt[ts(kc, 128), :])

        def phase_a(b):
            # Persistent per-batch PSUM accumulators. Heads are "pair-packed":
            # head h lives at partitions (h%2)*64..(h%2)*64+63.
            # Gram pair-matmuls write [128, 128] blocks per head pair j; the
            # useful data is the diagonal sub-blocks:
            #   acc2[r*64:+64, j, r*64:+64] = sum_n q_h^T k_h  (h = 2j + r)
            acc2 = accp.tile([128, 4, 128], F32, tag="acc")
            ssq_q = accp.tile([1, C], F32, tag="ssq_q")
            ssq_k = accp.tile([1, C], F32, tag="ssq_k")
            # x and v in channel-major layout (tiles per 128-channel group)
            xT = [
                vtp.tile([128, N], op_dt, tag="xt_cm", name=f"xtcm{g}", bufs=4)
                for g in range(4)
            ]
            vT = [vtp.tile([128, N], op_dt, tag="vt", name=f"vt{g}") for g in range(4)]

            # ---------------- Phase A: stream token chunks ----------------
            for ci in range(NCHUNK):
                xt = xin.tile([128, C], F32, tag="xt")
                nc.sync.dma_start(out=xt, in_=x[b, ts(ci, 128), :])
                xb = chunk.tile([128, C], op_dt, tag="xb", bufs=6)
                nc.gpsimd.tensor_copy(out=xb, in_=xt)
                # transpose x chunk into the channel-major batch buffer
                xtp = ps.tile([128, 4, 128], op_dt, tag="ps")
                for g in range(4):
                    nc.tensor.transpose(xtp[:, g, :], xb[:, ts(g, 128)], ident)
                for g in range(4):
                    nc.vector.tensor_copy(out=xT[g][:, ts(ci, 128)], in_=xtp[:, g, :])

                # q,k projection, token-major: stationary = xT chunk
                qp = ps.tile([128, C], F32, tag="ps")
                kp = ps.tile([128, C], F32, tag="ps")
                for kc in range(4):
                    for g, dst in enumerate((qp, kp)):
                        nc.tensor.matmul(
                            dst,
                            mm(xT[kc][:, ts(ci, 128)]),
                            mm(qkv_wT[kc][:, g * C : (g + 1) * C]),
                            start=(kc == 0),
                            stop=(kc == 3),
                        )

                qks = chunk.tile([128, 2, C], op_dt, tag="qks")
                nc.scalar.copy(out=qks[:, 0, :], in_=qp)
                nc.vector.tensor_copy(out=qks[:, 1, :], in_=kp)

                # squared q,k for the sum-of-squares accumulators
                sq = chunk.tile([128, 2, C], op_dt, tag="sq")
                nc.gpsimd.tensor_mul(out=sq, in0=qks, in1=qks)

                # attn Gram accumulation, one [128,128] matmul per head pair
                # (off-diagonal blocks are computed but unused). Two
                # independent accumulation groups per bank (partitions 0-63
                # and 64-127), each with exactly one start and stop.
                for j in range(4):
                    nc.tensor.matmul(
                        acc2[:, j, :],
                        qks[:, 0, ts(j, 2 * D)],
                        qks[:, 1, ts(j, 2 * D)],
                        start=(ci == 0 and j == 0),
                        stop=(ci == NCHUNK - 1 and j == 3),
                    )
                # sum-of-squares via ones-vector matmul
                nc.tensor.matmul(
                    ssq_q,
                    mm(ones_col),
                    mm(sq[:, 0, :]),
                    start=(ci == 0),
                    stop=(ci == NCHUNK - 1),
                )
                nc.tensor.matmul(
                    ssq_k,
                    mm(ones_col),
                    mm(sq[:, 1, :]),
                    start=(ci == 0),
                    stop=(ci == NCHUNK - 1),
                )

                # v projection, channel-major directly: stationary = v weight
                # block [cin, cout128], moving = xT 512-token slices. One
                # (nj, mc) sub-block per chunk to keep PSUM pool pressure flat;
                # group nj's inputs are ready once chunk 4*nj+3 is transposed.
                vjobs = []
                if ci >= 4:
                    vjobs.append((ci // 4 - 1, ci % 4))
                if ci == NCHUNK - 1:
                    vjobs += [(NCHUNK // 4 - 1, mc) for mc in range(4)]
                for nj, mc in vjobs:
                    vps = ps.tile([128, 512], F32, tag="ps")
                    for kc in range(4):
                        nc.tensor.matmul(
                            vps,
                            mm(qkv_wT[kc][:, 2 * C + mc * 128 : 2 * C + (mc + 1) * 128]),
                            mm(xT[kc][:, ts(nj, 512)]),
                            start=(kc == 0),
                            stop=(kc == 3),
                        )
                    nc.scalar.copy(out=vT[mc][:, ts(nj, 512)], in_=vps)

            # extract the PSUM accumulators immediately so the next batch can
            # reuse the accumulator banks while this batch's softmax waits
            attn = small.tile([128, 4, D], F32, tag="attn")
            nc.scalar.copy(out=attn[0:64], in_=acc2[0:64, :, 0:64])
            nc.scalar.copy(out=attn[64:128], in_=acc2[64:128, :, 64:128])
            ssq = small.tile([1, 2, C], F32, tag="ssq")
            nc.vector.tensor_copy(out=ssq[:, 0, :], in_=ssq_q)
            nc.vector.tensor_copy(out=ssq[:, 1, :], in_=ssq_k)
            return attn, ssq, vT

        def phase_b(b, attn, ssq, vT):
            # ---------------- Phase B: softmax + out + proj ----------------
            # Everything pair-packed: [128 partitions, 4 pair slots, 64].

            # inv norm = 1/max(sqrt(ssq), eps); fold temperature into q side
            nrm = small.tile([1, 2, H, D], F32, tag="nrm")
            nc.scalar.sqrt(out=nrm, in_=ssq.rearrange("p t (h d) -> p t h d", h=H))
            nc.vector.tensor_scalar_max(nrm, nrm, EPS)
            nc.vector.reciprocal(out=nrm, in_=nrm)
            temp_bc = bass.AP(
                tensor=temp_sb.tensor,
                offset=temp_sb.offset,
                ap=[list(temp_sb.ap[0]), [1, H], [0, D]],
            )
            nc.vector.tensor_tensor(
                out=nrm[:, 0], in0=nrm[:, 0], in1=temp_bc, op=mybir.AluOpType.mult
            )

            # alpha[p, j] = inv_q[ch] * temp for channel ch=(2j + p//64)*64 + p%64
            # via 4 tiny PE transposes of [1,128] slices -> [128,1] columns
            alpha_ps = ps.tile([128, 4], F32, tag="ps")
            for j in range(4):
                nc.tensor.transpose(
                    alpha_ps[:, j : j + 1],
                    nrm[0:1, 0].rearrange("p h d -> p (h d)")[:, ts(j, 128)],
                    id1_f32,
                )
            alpha = small.tile([128, 4], F32, tag="alpha")
            nc.vector.tensor_copy(out=alpha, in_=alpha_ps)

            # inv_k broadcast, pair-packed: partitions 0-63 get even heads,
            # 64-127 get odd heads (two K=1 ones-matmuls)
            ikb_ps = ps.tile([128, 4, D], F32, tag="ps")
            nrm_k = nrm[:, 1]  # [1, H, D]
            nc.tensor.matmul(
                ikb_ps[0:64], ones_f32[:, 0:64], nrm_k[:, 0::2, :],
                start=True, stop=True,
            )
            nc.tensor.matmul(
                ikb_ps[64:128], ones_f32[:, 0:64], nrm_k[:, 1::2, :],
                start=True, stop=True,
            )
            ikb = small.tile([128, 4, D], F32, tag="ikb")
            nc.vector.tensor_copy(out=ikb, in_=ikb_ps)

            # z = gram * inv_k (free axis) * alpha (per partition+slot)
            nc.vector.tensor_mul(out=attn, in0=attn, in1=ikb)
            alpha_bc = bass.AP(
                tensor=alpha.tensor,
                offset=alpha.offset,
                ap=[list(alpha.ap[0]), list(alpha.ap[1]), [0, D]],
            )
            nc.vector.tensor_tensor(
                out=attn, in0=attn, in1=alpha_bc, op=mybir.AluOpType.mult
            )

            # softmax over the last axis (per head)
            mx = small.tile([128, 4], F32, tag="mx")
            nc.vector.tensor_reduce(
                out=mx, in_=attn, axis=mybir.AxisListType.X,
                op=mybir.AluOpType.max, negate=True,
            )
            mx_bc = bass.AP(
                tensor=mx.tensor, offset=mx.offset,
                ap=[list(mx.ap[0]), list(mx.ap[1]), [0, D]],
            )
            nc.vector.tensor_tensor(
                out=attn, in0=attn, in1=mx_bc, op=mybir.AluOpType.add
            )
            ex = small.tile([128, 4, D], F32, tag="ex")
            nc.scalar.activation(
                out=ex, in_=attn, func=mybir.ActivationFunctionType.Exp
            )
            rs = small.tile([128, 4], F32, tag="rs")
            nc.vector.tensor_reduce(
                out=rs, in_=ex, axis=mybir.AxisListType.X, op=mybir.AluOpType.add
            )
            nc.vector.reciprocal(out=rs, in_=rs)
            probs = small.tile([128, 4, D], op_dt, tag="probs")
            rs_bc = bass.AP(
                tensor=rs.tensor, offset=rs.offset,
                ap=[list(rs.ap[0]), list(rs.ap[1]), [0, D]],
            )
            nc.vector.tensor_tensor(
                out=probs, in0=ex, in1=rs_bc, op=mybir.AluOpType.mult
            )

            # transpose probs (per head) -> attnT, same pair-packed layout
            atp = ps.tile([128, 4, D], op_dt, tag="ps")
            for h in range(H):
                r = h % 2
                sl = slice(r * 64, r * 64 + 64)
                nc.tensor.transpose(
                    atp[sl, h // 2, :],
                    probs[sl, h // 2, :],
                    ident[sl, sl],
                )
            attnT = small.tile([128, 4, D], op_dt, tag="attnT")
            nc.vector.tensor_copy(out=attnT, in_=atp)

            # out = attn @ v (channel-major), then proj back to token-major
            for nj in range(N // 512):
                # separate per-group tiles so each proj matmul only waits on
                # the one outT group it actually reads
                outT = [
                    outp.tile([128, 512], op_dt, tag=f"outT{g}", name=f"outT{g}")
                    for g in range(4)
                ]
                for g in range(4):
                    ops = ps.tile([128, 512], F32, tag="ps")
                    for r in range(2):
                        sl = slice(r * 64, r * 64 + 64)
                        nc.tensor.matmul(
                            ops[sl, :],
                            mm(attnT[sl, g, :]),
                            mm(vT[g][sl, ts(nj, 512)]),
                            start=True,
                            stop=True,
                        )
                    nc.scalar.copy(out=outT[g], in_=ops)
                for t4 in range(4):
                    ypt = ps.tile([128, 512], F32, tag="ps")
                    for kc in range(4):
                        nc.tensor.matmul(
                            ypt,
                            mm(outT[kc][:, ts(t4, 128)]),
                            mm(proj_wT[kc]),
                            start=(kc == 0),
                            stop=(kc == 3),
                        )
                    ysb = yp.tile([128, C], F32, tag="ysb")
                    nc.vector.tensor_add(out=ysb, in0=ypt, in1=bias_bc)
                    nc.sync.dma_start(
                        out=out[b, nj * 512 + t4 * 128 : nj * 512 + (t4 + 1) * 128, :],
                        in_=ysb,
                    )

        # software-pipeline the batches: emit batch b's softmax/out/proj
        # after batch b+1's phase A so the PE queue stays dense while the
        # small softmax chain runs on ACT/DVE
        pending = None
        for b in range(BL):
            ctx = phase_a(b)
            if pending is not None:
                phase_b(*pending)
            pending = (b, *ctx)
        phase_b(*pending)

        accp.release()
        ps.release()
        yp.release()
        outp.release()
        small.release()
        vtp.release()
        chunk.release()
        xin.release()
        consts.release()

    legalize_waits(nc)
    return nc


def build_trivial_bass():
    """Minimal kernel used by the benchmark harness to measure the
    per-dispatch floor (axon round trip + runtime overhead)."""
    nc = bass.Bass(trn_type="TRN2")
    inp = nc.dram_tensor("inp", [128, 512], F32, kind="ExternalInput")
    outp = nc.dram_tensor("outp", [128, 512], F32, kind="ExternalOutput")
    with TileContext(nc) as tc:
        with tc.tile_pool(name="p", bufs=1) as pool:
            s = pool.tile([128, 512], F32)
            nc.sync.dma_start(out=s, in_=inp[:, :])
            nc.sync.dma_start(out=outp[:, :], in_=s)
    legalize_waits(nc)
    return nc


_NC_CACHE = {}


def kernel(x, qkv_w, temperature, proj_w, proj_b, _want_trace=False, _trace_kwargs=None):
    x = np.ascontiguousarray(x, dtype=np.float32)
    key = MM_MODE
    if key not in _NC_CACHE:
        _NC_CACHE[key] = build_bass()
    nc = _NC_CACHE[key]

    temp_flat = np.ascontiguousarray(np.asarray(temperature, np.float32).reshape(H))
    if MM_MODE == "bf16":
        import ml_dtypes

        w_dt = ml_dtypes.bfloat16
    else:
        w_dt = np.float32
    qkv_wt = np.ascontiguousarray(np.asarray(qkv_w, np.float32).T.astype(w_dt))
    proj_wt = np.ascontiguousarray(np.asarray(proj_w, np.float32).T.astype(w_dt))
    in_maps = []
    for i in range(NCORES):
        in_maps.append(
            {
                "x": np.ascontiguousarray(x[i * BL : (i + 1) * BL]),
                "qkv_wt": qkv_wt,
                "temperature": temp_flat,
                "proj_wt": proj_wt,
                "proj_b": np.ascontiguousarray(proj_b, np.float32),
            }
        )
    res = run_bass_kernel_spmd(
        nc,
        in_maps,
        core_ids=list(range(NCORES)),
        trace=_want_trace,
        **(_trace_kwargs or {}),
    )
    y = np.concatenate([res.results[i]["out"] for i in range(NCORES)], axis=0)
    if _want_trace:
        return y, res
    return y



# revision 2
# speedup vs baseline: 1.4465x; 1.4465x over previous
"""Trainium2 Bass kernel for nn_CABlock (channel attention / XCA block).

Reference computation (per batch b):
  qkv = x @ qkv_w.T                      # [N, 3C], token-major
  q,k,v per head: [d=64, N] channel-major after reshape/transpose
  q,k l2-normalized over N; attn = softmax((q @ k.T) * temperature, axis=-1)
  out = attn @ v  -> [N, C];  y = out @ proj_w.T + proj_b

Key restructure vs the reference:
  * l2norm commutes with the bilinear form:
      logits = diag(inv_q) @ (q_raw @ k_raw.T) @ diag(inv_k) * temp
  * per head h we run ONE [128,128] Gram matmul with stationary=moving=
    (q_h | k_h) packed along the free axis.  The [0:64,64:128] block is
    q^T k and the diagonals of the [0:64,0:64] / [64:128,64:128] blocks are
    the q/k sums of squares -- the l2 norms come for free, no elementwise
    squaring or separate ones-matmul reduction needed.
  * x arrives from the host already transposed to channel-major bf16
    ([C, N] per batch), so no on-device transposes of x are needed: the
    q/k projection contracts channels with x-chunks stationary, the v
    projection emits v channel-major directly, and the same layout feeds
    the final out-projection.
  * attn @ v uses a block-diagonal packed attn^T stationary ([128,128] per
    head pair) so the PE runs with all 128 contraction rows active.
  * phase-B softmax scalar chain of batch b is interleaved into the first
    few chunks of batch b+1's phase A so the PE never waits on it.

Sharding: data-parallel over batch B=16 across 8 cores (2 batches/core).
No collectives needed.
"""

import os
import sys

import numpy as np

for _p in ("/opt/trn_rl_repo", "/root/.axon_site/_ro/trn_rl_repo"):
    if os.path.isdir(_p) and _p not in sys.path:
        sys.path.insert(0, _p)

import concourse.bass as bass  # noqa: E402
from concourse import mybir  # noqa: E402
from concourse.bass import ts  # noqa: E402
from concourse.bass_utils import run_bass_kernel_spmd  # noqa: E402
from concourse.masks import make_identity  # noqa: E402
from concourse.tile import TileContext  # noqa: E402

B, N, C = 16, 4096, 512
H, D = 8, 64
C3 = 3 * C
NCORES = 8
BL = B // NCORES  # batches per core
EPS = 1e-12
NCHUNK = N // 128  # 32 token chunks per batch
F32 = mybir.dt.float32
BF16 = mybir.dt.bfloat16

MM_MODE = "bf16"  # kept for test.py compatibility


def legalize_waits(nc):
    """Walrus in this environment rejects instructions carrying more than one
    semaphore wait ("Too many sync wait commands"), and rejects sem-ge waits
    on Drain instructions entirely. Tile emits both. Hoist the offending
    waits onto standalone EventSemaphore instructions inserted immediately
    before the instruction on the same engine queue — semantically identical
    (the engine executes the waits, then the instruction)."""
    n_new = 0
    for bb in nc.main_func.blocks:
        il = bb.instructions
        new_list = []
        for ins in il:
            si = ins.sync_info
            waits = list(si.on_wait) if si is not None and si.on_wait else []
            if waits:
                tname = type(ins).__name__
                no_wait_slots = tname in ("InstDrain", "InstDmaTransposeAnt") or (
                    getattr(ins, "opcode", "") in ("Drain", "DmaTransposeAnt")
                )
                keep_budget = 0 if no_wait_slots else 1
                if len(waits) > keep_budget:
                    hoist, keep = waits[:-keep_budget] if keep_budget else waits, (
                        waits[-keep_budget:] if keep_budget else []
                    )
                    for w in hoist:
                        ev = mybir.InstEventSemaphore(
                            name=f"{ins.name}-hoistw{n_new}",
                            ins=[],
                            outs=[],
                            engine=ins.engine,
                            sync_info=mybir.SyncInfo(on_wait=[w], on_update=[]),
                        )
                        new_list.append(ev)
                        n_new += 1
                    ins.sync_info = mybir.SyncInfo(
                        on_wait=keep, on_update=list(si.on_update or [])
                    )
            new_list.append(ins)
        il.clear()
        il.extend(new_list)
    return n_new


def build_bass():
    nc = bass.Bass(trn_type="TRN2")
    # x arrives channel-major bf16 from the host wrapper: [BL, C, N]
    x = nc.dram_tensor("x", [BL, C, N], BF16, kind="ExternalInput")
    qkv_wt = nc.dram_tensor("qkv_wt", [C, C3], BF16, kind="ExternalInput")
    temp = nc.dram_tensor("temperature", [H], F32, kind="ExternalInput")
    proj_wt = nc.dram_tensor("proj_wt", [C, C], BF16, kind="ExternalInput")
    proj_b = nc.dram_tensor("proj_b", [C], F32, kind="ExternalInput")
    out = nc.dram_tensor("out", [BL, N, C], F32, kind="ExternalOutput")

    with TileContext(nc) as tc:
        consts = tc.alloc_tile_pool(name="consts", bufs=1)
        xtp = tc.alloc_tile_pool(name="xtp", bufs=2)
        vtp = tc.alloc_tile_pool(name="vtp", bufs=2)
        qksp = tc.alloc_tile_pool(name="qksp", bufs=3)
        small = tc.alloc_tile_pool(name="small", bufs=2)
        outp = tc.alloc_tile_pool(name="outp", bufs=2)
        yp = tc.alloc_tile_pool(name="yp", bufs=3)
        ps = tc.alloc_tile_pool(name="ps", bufs=6, space="PSUM")
        accp = tc.alloc_tile_pool(name="accp", bufs=1, space="PSUM")

        # ---- constants ----
        ident_b = consts.tile([128, 128], BF16)
        make_identity(nc, ident_b)
        ident_f = consts.tile([128, 128], F32)
        make_identity(nc, ident_f)
        ones_col = consts.tile([128, 1], BF16)
        nc.vector.memset(ones_col, 1.0)
        ones_row = consts.tile([1, 128], BF16)
        nc.vector.memset(ones_row, 1.0)
        ones_f32 = consts.tile([1, 128], F32)
        nc.vector.memset(ones_f32, 1.0)

        temp_sb = consts.tile([1, H], F32)
        nc.sync.dma_start(out=temp_sb, in_=temp[:])

        # proj bias: load [1, C] then broadcast to all partitions via a
        # K=1 ones-matmul on the PE (out[p, c] = 1 * bias[c])
        bias_row = consts.tile([1, C], F32)
        nc.sync.dma_start(out=bias_row, in_=proj_b[:])
        bias_ps = ps.tile([128, C], F32, tag="ps")
        nc.tensor.matmul(bias_ps, ones_f32, bias_row, start=True, stop=True)
        bias_bc = consts.tile([128, C], F32)
        nc.vector.tensor_copy(out=bias_bc, in_=bias_ps)

        # weights: [cin, cout] bf16, plain DMA loads
        qkv_wT = [consts.tile([128, C3], BF16, tag=f"qkvwT{i}", name=f"qkvwT{i}") for i in range(4)]
        proj_wT = [consts.tile([128, C], BF16, tag=f"projwT{i}", name=f"projwT{i}") for i in range(4)]
        for kc in range(4):
            nc.sync.dma_start(out=qkv_wT[kc], in_=qkv_wt[ts(kc, 128), :])
            nc.sync.dma_start(out=proj_wT[kc], in_=proj_wt[ts(kc, 128), :])

        def phase_a(b, interleave):
            """Stream one batch through qkv projection + Gram accumulation.
            ``interleave[i]`` (previous batch's phase-B scalar stages) is
            emitted after chunk i so its ACT/DVE work hides under PE time."""
            xT = [xtp.tile([128, N], BF16, tag=f"xt{g}", name=f"xt{g}") for g in range(4)]
            # load channel-major x in 512-token granules so chunk 0 only
            # waits for the first granule of each channel group
            for j in range(N // 512):
                for g in range(4):
                    nc.sync.dma_start(
                        out=xT[g][:, ts(j, 512)], in_=x[b, ts(g, 128), ts(j, 512)]
                    )
            vT = [vtp.tile([128, N], BF16, tag=f"vt{g}", name=f"vt{g}") for g in range(4)]

            # Gram accumulator: acc2[:, h, :] = (q_h|k_h)^T (q_h|k_h) summed
            # over all token chunks. 2 PSUM banks; per-bank start/stop.
            acc2 = accp.tile([128, H, 128], F32, tag="acc")
            qks_tiles = {}

            def emit_gram(cj):
                qks = qks_tiles.pop(cj)
                for h in range(H):
                    nc.tensor.matmul(
                        acc2[:, h, :],
                        qks[:, h],
                        qks[:, h],
                        start=(cj == 0 and h % 4 == 0),
                        stop=(cj == NCHUNK - 1 and h % 4 == 3),
                    )

            for ci in range(NCHUNK):
                # q,k projection for this token chunk (stationary = x chunk)
                qp = ps.tile([128, C], F32, tag="ps")
                kp = ps.tile([128, C], F32, tag="ps")
                for kc in range(4):
                    nc.tensor.matmul(
                        qp, xT[kc][:, ts(ci, 128)], qkv_wT[kc][:, 0:C],
                        start=(kc == 0), stop=(kc == 3),
                    )
                for kc in range(4):
                    nc.tensor.matmul(
                        kp, xT[kc][:, ts(ci, 128)], qkv_wT[kc][:, C : 2 * C],
                        start=(kc == 0), stop=(kc == 3),
                    )
                # interleaved per-head (q_h | k_h) packing for the Gram step
                qks = qksp.tile([128, H, 2, 64], BF16, tag="qks")
                nc.scalar.copy(
                    out=qks[:, :, 0, :], in_=qp.rearrange("p (h d) -> p h d", h=H)
                )
                nc.vector.tensor_copy(
                    out=qks[:, :, 1, :], in_=kp.rearrange("p (h d) -> p h d", h=H)
                )
                qks_tiles[ci] = qks

                # v projection, channel-major directly (one 128-out-ch x
                # 512-token block per chunk covers the whole batch evenly)
                nj, mc = ci // 4, ci % 4
                vps = ps.tile([128, 512], F32, tag="ps")
                for kc in range(4):
                    nc.tensor.matmul(
                        vps,
                        qkv_wT[kc][:, 2 * C + mc * 128 : 2 * C + (mc + 1) * 128],
                        xT[kc][:, ts(nj, 512)],
                        start=(kc == 0), stop=(kc == 3),
                    )
                nc.scalar.copy(out=vT[mc][:, ts(nj, 512)], in_=vps)

                # Gram for the previous chunk (its qks copies are done by now)
                if ci >= 1:
                    emit_gram(ci - 1)
                if ci < len(interleave):
                    interleave[ci]()
            emit_gram(NCHUNK - 1)

            # extract logits + masked diagonals immediately so the next batch
            # can reuse the accumulator banks
            attn = small.tile([64, H, 64], F32, tag="attn")
            nc.scalar.copy(out=attn, in_=acc2[0:64, :, 64:128])
            masked = small.tile([128, H, 64], BF16, tag="masked")
            iq = ident_f[0:64, 0:64]
            iq_bc = bass.AP(
                tensor=iq.tensor, offset=iq.offset,
                ap=[list(iq.ap[0]), [0, H], list(iq.ap[1])],
            )
            nc.vector.tensor_tensor(
                out=masked[0:64], in0=acc2[0:64, :, 0:64], in1=iq_bc,
                op=mybir.AluOpType.mult,
            )
            ik = ident_f[64:128, 64:128]
            ik_bc = bass.AP(
                tensor=ik.tensor, offset=ik.offset,
                ap=[list(ik.ap[0]), [0, H], list(ik.ap[1])],
            )
            nc.vector.tensor_tensor(
                out=masked[64:128], in0=acc2[64:128, :, 64:128], in1=ik_bc,
                op=mybir.AluOpType.mult,
            )
            return attn, masked, vT

        def phase_b_stages(b, attn, masked, vT):
            """Returns ([s1..s5] scalar stages to interleave into the next
            batch's phase A, final_block with the attn@v + proj matmuls)."""
            ssqd = small.tile([64, H], F32, tag="ssqd")
            invq = small.tile([64, H], F32, tag="invq")
            krow = small.tile([1, H, 64], F32, tag="krow")
            krow_b = small.tile([1, H, 64], BF16, tag="krowb")
            ikb = small.tile([64, H, 64], F32, tag="ikb")
            probs = small.tile([64, H, 64], BF16, tag="probs")
            bd = small.tile([128, 4, 128], BF16, tag="bd")
            mx = small.tile([64, H], F32, tag="mx")
            ex = small.tile([64, H, 64], F32, tag="ex")
            rs = small.tile([64, H], F32, tag="rs")

            def s1():
                # inv_q = 1/max(sqrt(ssq_q), eps) per (d, h), partition-major
                nc.vector.tensor_reduce(
                    out=ssqd, in_=masked[0:64], axis=mybir.AxisListType.X,
                    op=mybir.AluOpType.add,
                )
                nc.scalar.sqrt(out=invq, in_=ssqd)
                nc.vector.tensor_scalar_max(invq, invq, EPS)
                nc.vector.reciprocal(out=invq, in_=invq)
                # ssq_k as a [1, H*64] row: ones^T @ masked k-half
                krow_ps = ps.tile([1, H, 64], F32, tag="ps")
                nc.tensor.matmul(
                    krow_ps, ones_col[64:128, :], masked[64:128],
                    start=True, stop=True,
                )
                nc.vector.tensor_copy(out=krow, in_=krow_ps)

            def s2():
                # inv_k row, temperature folded in (uniform over d AND e per h)
                nc.scalar.sqrt(out=krow, in_=krow)
                nc.vector.tensor_scalar_max(krow, krow, EPS)
                nc.vector.reciprocal(out=krow, in_=krow)
                temp_bc = bass.AP(
                    tensor=temp_sb.tensor, offset=temp_sb.offset,
                    ap=[list(temp_sb.ap[0]), [1, H], [0, D]],
                )
                nc.vector.tensor_tensor(
                    out=krow_b, in0=krow, in1=temp_bc, op=mybir.AluOpType.mult
                )

            def s3():
                # broadcast inv_k*temp over the 64 d-partitions via K=1 matmul
                ikb_ps = ps.tile([64, H, 64], F32, tag="ps")
                nc.tensor.matmul(
                    ikb_ps, ones_row[:, 0:64],
                    krow_b.rearrange("p h d -> p (h d)"),
                    start=True, stop=True,
                )
                nc.vector.tensor_copy(out=ikb, in_=ikb_ps)
                nc.vector.tensor_mul(out=attn, in0=attn, in1=ikb)
                invq_bc = bass.AP(
                    tensor=invq.tensor, offset=invq.offset,
                    ap=[list(invq.ap[0]), list(invq.ap[1]), [0, D]],
                )
                nc.vector.tensor_tensor(
                    out=attn, in0=attn, in1=invq_bc, op=mybir.AluOpType.mult
                )

            def s4():
                # softmax over the last axis (per head)
                nc.vector.tensor_reduce(
                    out=mx, in_=attn, axis=mybir.AxisListType.X,
                    op=mybir.AluOpType.max, negate=True,
                )
                mx_bc = bass.AP(
                    tensor=mx.tensor, offset=mx.offset,
                    ap=[list(mx.ap[0]), list(mx.ap[1]), [0, D]],
                )
                nc.vector.tensor_tensor(
                    out=attn, in0=attn, in1=mx_bc, op=mybir.AluOpType.add
                )
                nc.scalar.activation(
                    out=ex, in_=attn, func=mybir.ActivationFunctionType.Exp
                )
                nc.vector.tensor_reduce(
                    out=rs, in_=ex, axis=mybir.AxisListType.X,
                    op=mybir.AluOpType.add,
                )
                nc.vector.reciprocal(out=rs, in_=rs)
                rs_bc = bass.AP(
                    tensor=rs.tensor, offset=rs.offset,
                    ap=[list(rs.ap[0]), list(rs.ap[1]), [0, D]],
                )
                nc.vector.tensor_tensor(
                    out=probs, in0=ex, in1=rs_bc, op=mybir.AluOpType.mult
                )

            def s5():
                # block-diagonal packed attn^T: pair g holds head 2g at
                # [0:64, 0:64] and head 2g+1 at [64:128, 64:128]
                atp = ps.tile([128, 4, 64], BF16, tag="ps")
                for h in range(H):
                    r, g = h % 2, h // 2
                    nc.tensor.transpose(
                        atp[r * 64 : r * 64 + 64, g, :],
                        probs[:, h, :],
                        ident_b[0:64, 0:64],
                    )
                nc.vector.memset(bd, 0.0)
                nc.scalar.copy(out=bd[0:64, :, 0:64], in_=atp[0:64])
                nc.scalar.copy(out=bd[64:128, :, 64:128], in_=atp[64:128])

            def final_block():
                for nj in range(N // 512):
                    outT = [
                        outp.tile([128, 512], BF16, tag=f"outT{g}", name=f"outT{g}")
                        for g in range(4)
                    ]
                    for g in range(4):
                        ops = ps.tile([128, 512], F32, tag="ps")
                        nc.tensor.matmul(
                            ops, bd[:, g, :], vT[g][:, ts(nj, 512)],
                            start=True, stop=True,
                        )
                        nc.scalar.copy(out=outT[g], in_=ops)
                    for t4 in range(4):
                        ypt = ps.tile([128, C], F32, tag="ps")
                        for kc in range(4):
                            nc.tensor.matmul(
                                ypt, outT[kc][:, ts(t4, 128)], proj_wT[kc],
                                start=(kc == 0), stop=(kc == 3),
                            )
                        ysb = yp.tile([128, C], F32, tag="ysb")
                        nc.vector.tensor_add(out=ysb, in0=ypt, in1=bias_bc)
                        nc.sync.dma_start(
                            out=out[b, nj * 512 + t4 * 128 : nj * 512 + (t4 + 1) * 128, :],
                            in_=ysb,
                        )

            return [s1, s2, s3, s4, s5], final_block

        pending = None
        for b in range(BL):
            stages = pending[0] if pending else []
            ctx = phase_a(b, stages)
            if pending is not None:
                pending[1]()
            pending = phase_b_stages(b, *ctx)
        for s in pending[0]:
            s()
        pending[1]()

        accp.release()
        ps.release()
        yp.release()
        outp.release()
        small.release()
        qksp.release()
        vtp.release()
        xtp.release()
        consts.release()

    legalize_waits(nc)
    return nc


def build_trivial_bass():
    """Minimal kernel used by the benchmark harness to measure the
    per-dispatch floor (axon round trip + runtime overhead)."""
    nc = bass.Bass(trn_type="TRN2")
    inp = nc.dram_tensor("inp", [128, 512], F32, kind="ExternalInput")
    outp = nc.dram_tensor("outp", [128, 512], F32, kind="ExternalOutput")
    with TileContext(nc) as tc:
        with tc.tile_pool(name="p", bufs=1) as pool:
            s = pool.tile([128, 512], F32)
            nc.sync.dma_start(out=s, in_=inp[:, :])
            nc.sync.dma_start(out=outp[:, :], in_=s)
    legalize_waits(nc)
    return nc


_NC_CACHE = {}


def make_in_maps(x, qkv_w, temperature, proj_w, proj_b):
    import ml_dtypes

    bf = ml_dtypes.bfloat16
    x = np.asarray(x, np.float32)
    temp_flat = np.ascontiguousarray(np.asarray(temperature, np.float32).reshape(H))
    qkv_wt = np.ascontiguousarray(np.asarray(qkv_w, np.float32).T.astype(bf))
    proj_wt = np.ascontiguousarray(np.asarray(proj_w, np.float32).T.astype(bf))
    pb = np.ascontiguousarray(np.asarray(proj_b, np.float32))
    xb = x.astype(bf)
    in_maps = []
    for i in range(NCORES):
        xcm = np.ascontiguousarray(xb[i * BL : (i + 1) * BL].transpose(0, 2, 1))
        in_maps.append(
            {
                "x": xcm,
                "qkv_wt": qkv_wt,
                "temperature": temp_flat,
                "proj_wt": proj_wt,
                "proj_b": pb,
            }
        )
    return in_maps


def kernel(x, qkv_w, temperature, proj_w, proj_b, _want_trace=False, _trace_kwargs=None):
    key = MM_MODE
    if key not in _NC_CACHE:
        _NC_CACHE[key] = build_bass()
    nc = _NC_CACHE[key]

    in_maps = make_in_maps(x, qkv_w, temperature, proj_w, proj_b)
    res = run_bass_kernel_spmd(
        nc,
        in_maps,
        core_ids=list(range(NCORES)),
        trace=_want_trace,
        **(_trace_kwargs or {}),
    )
    y = np.concatenate([res.results[i]["out"] for i in range(NCORES)], axis=0)
    if _want_trace:
        return y, res
    return y


# revision 6
# speedup vs baseline: 2.2319x; 1.5430x over previous
"""Trainium2 Bass kernel for nn_CABlock (channel attention / XCA block).

Reference computation (per batch b):
  qkv = x @ qkv_w.T                      # [N, 3C], token-major
  q,k,v per head: [d=64, N] channel-major after reshape/transpose
  q,k l2-normalized over N; attn = softmax((q @ k.T) * temperature, axis=-1)
  out = attn @ v  -> [N, C];  y = out @ proj_w.T + proj_b

Numerics / restructure:
  * l2norm commutes with the bilinear form:
      logits = diag(inv_q) @ (q_raw @ k_raw.T) @ diag(inv_k) * temp
    so any uniform scaling of q/k (from fp8 pre-scaling) cancels exactly.
  * q/k projection and the token Gram run in fp8 (e4m3) DoubleRow mode:
    256-deep contraction at 0.5 PE cycles per output column, 4x bf16
    throughput.  Softmax over the tiny normalized logits washes out the
    quantization noise (validated: ~4.5e-3 rel err end to end).
  * v projection runs as a 3-term residual-corrected fp8 product
      v = [W8@x8 + W8@xr8 + Wr8@x8] / (SX*SWV)
    with x8=fp8(SX*x), xr8=fp8(SX*x-x8), W8=fp8(SWV*W), Wr8=fp8(SWV*W-W8),
    all host-prepared.  Dropped term Wr*xr is ~0.1%^2.  attn@v and the
    final projection stay bf16 (fp8 there fails the error budget).
  * per head h ONE Gram matmul with stationary=moving=(q_h | k_h) packed
    along the free axis: the [0:64,64:128] block is q^T k and the
    diagonals of the other blocks are the q/k sums of squares -- norms
    come for free.
  * x arrives host-transposed channel-major (DoubleRow k-tile packed), so
    no on-device transposes of x are needed anywhere.
  * attn @ v uses a block-diagonal packed attn^T stationary so the PE runs
    all 128 contraction rows; the final block is software-pipelined
    (attn@v two tiles ahead of the projection) and the last batch's
    softmax scalar chain is interleaved into the previous batch's final
    block so the PE never waits on the ACT/DVE chain.

Sharding: data-parallel over batch B=16 across 8 cores (2 batches/core).
No collectives needed.
"""

import os
import sys

import numpy as np

for _p in ("/opt/trn_rl_repo", "/root/.axon_site/_ro/trn_rl_repo"):
    if os.path.isdir(_p) and _p not in sys.path:
        sys.path.insert(0, _p)

import concourse.bass as bass  # noqa: E402
from concourse import mybir  # noqa: E402
from concourse.bass import ts  # noqa: E402
from concourse.bass_utils import run_bass_kernel_spmd  # noqa: E402
from concourse.masks import make_identity  # noqa: E402
from concourse.tile import TileContext  # noqa: E402

B, N, C = 16, 4096, 512
H, D = 8, 64
C3 = 3 * C
NCORES = 8
BL = B // NCORES  # batches per core
EPS = 1e-12
NCHUNK = N // 128  # 32 token chunks per batch
NPAIR = NCHUNK // 2
F32 = mybir.dt.float32
BF16 = mybir.dt.bfloat16
F8 = mybir.dt.float8e4
DR = mybir.MatmulPerfMode.DoubleRow

# fp8 pre-scales (powers of two; exactly cancelled on-device). Device
# float8e4 is IEEE e4m3: max finite 240, NO saturation (overflow -> inf),
# so scales keep every fp8 value comfortably under ~100.
SX = 2.0  # x  (|2x| <~ 11)
SWQ = 16.0  # q,k weight columns (|32*q| <~ 80 for the fp8 qks re-quant)
SWV = 256.0  # v weight columns (|256*wv| <~ 26)

MM_MODE = "bf16"  # kept for test.py compatibility


def legalize_waits(nc):
    """Walrus in this environment rejects instructions carrying more than one
    semaphore wait ("Too many sync wait commands"), and rejects sem-ge waits
    on Drain instructions entirely. Tile emits both. Hoist the offending
    waits onto standalone EventSemaphore instructions inserted immediately
    before the instruction on the same engine queue — semantically identical
    (the engine executes the waits, then the instruction)."""
    n_new = 0
    for bb in nc.main_func.blocks:
        il = bb.instructions
        new_list = []
        for ins in il:
            si = ins.sync_info
            waits = list(si.on_wait) if si is not None and si.on_wait else []
            if waits:
                tname = type(ins).__name__
                no_wait_slots = tname in ("InstDrain", "InstDmaTransposeAnt") or (
                    getattr(ins, "opcode", "") in ("Drain", "DmaTransposeAnt")
                )
                keep_budget = 0 if no_wait_slots else 1
                if len(waits) > keep_budget:
                    hoist, keep = waits[:-keep_budget] if keep_budget else waits, (
                        waits[-keep_budget:] if keep_budget else []
                    )
                    for w in hoist:
                        ev = mybir.InstEventSemaphore(
                            name=f"{ins.name}-hoistw{n_new}",
                            ins=[],
                            outs=[],
                            engine=ins.engine,
                            sync_info=mybir.SyncInfo(on_wait=[w], on_update=[]),
                        )
                        new_list.append(ev)
                        n_new += 1
                    ins.sync_info = mybir.SyncInfo(
                        on_wait=keep, on_update=list(si.on_update or [])
                    )
            new_list.append(ins)
        il.clear()
        il.extend(new_list)
    return n_new


def build_bass():
    nc = bass.Bass(trn_type="TRN2")
    # channel-major, DoubleRow k-tile packed: [b, kc2, k, t, n] = channel
    # kc2*256 + t*128 + k of batch b (value pre-scaled by SX, fp8)
    xq8 = nc.dram_tensor("xq8", [BL, 2, 128, 2, N], F8, kind="ExternalInput")
    xr8 = nc.dram_tensor("xr8", [BL, 2, 128, 2, N], F8, kind="ExternalInput")
    # q,k weight cols (x SWQ): [kc2, k, t, j] = qkv_wt[kc2*256+t*128+k, j]
    wqk8 = nc.dram_tensor("wqk8", [2, 128, 2, 2 * C], F8, kind="ExternalInput")
    wv8 = nc.dram_tensor("wv8", [2, 128, 2, C], F8, kind="ExternalInput")
    wvr8 = nc.dram_tensor("wvr8", [2, 128, 2, C], F8, kind="ExternalInput")
    temp = nc.dram_tensor("temperature", [H], F32, kind="ExternalInput")
    proj_wt = nc.dram_tensor("proj_wt", [C, C], BF16, kind="ExternalInput")
    proj_b = nc.dram_tensor("proj_b", [C], F32, kind="ExternalInput")
    out = nc.dram_tensor("out", [BL, N, C], F32, kind="ExternalOutput")

    with TileContext(nc) as tc:
        consts = tc.alloc_tile_pool(name="consts", bufs=1)
        xtp = tc.alloc_tile_pool(name="xtp", bufs=2)
        vtp = tc.alloc_tile_pool(name="vtp", bufs=2)
        qksp = tc.alloc_tile_pool(name="qksp", bufs=4)
        small = tc.alloc_tile_pool(name="small", bufs=2)
        outp = tc.alloc_tile_pool(name="outp", bufs=3)
        yp = tc.alloc_tile_pool(name="yp", bufs=3)
        ps = tc.alloc_tile_pool(name="ps", bufs=6, space="PSUM")
        accp = tc.alloc_tile_pool(name="accp", bufs=1, space="PSUM")

        # ---- tiny constants (cheap DMAs first) ----
        temp_sb = consts.tile([1, H], F32)
        nc.sync.dma_start(out=temp_sb, in_=temp[:])
        bias_row = consts.tile([1, C], F32)
        nc.sync.dma_start(out=bias_row, in_=proj_b[:])
        ident_b = consts.tile([128, 128], BF16)
        make_identity(nc, ident_b)
        ident_f = consts.tile([128, 128], F32)
        make_identity(nc, ident_f)
        ones_col = consts.tile([128, 1], BF16)
        nc.vector.memset(ones_col, 1.0)
        ones_row = consts.tile([1, 128], BF16)
        nc.vector.memset(ones_row, 1.0)
        ones_f32 = consts.tile([1, 128], F32)
        nc.vector.memset(ones_f32, 1.0)

        # ---- batch-0 x granule j=0 before the bulk weight loads so chunk 0
        # can start as early as possible ----
        def make_x_tiles():
            xq = [xtp.tile([128, 2, N], F8, tag=f"xq{g}", name=f"xq{g}") for g in range(2)]
            xr = [xtp.tile([128, 2, N], F8, tag=f"xr{g}", name=f"xr{g}") for g in range(2)]
            return xq, xr

        def emit_x_granule(b, xq, xr, j):
            for g in range(2):
                nc.sync.dma_start(out=xq[g][:, :, ts(j, 512)], in_=xq8[b, g, :, :, ts(j, 512)])
                nc.sync.dma_start(out=xr[g][:, :, ts(j, 512)], in_=xr8[b, g, :, :, ts(j, 512)])

        x0 = make_x_tiles()
        emit_x_granule(0, x0[0], x0[1], 0)

        # ---- weights needed by phase A ----
        wqk = [consts.tile([128, 2, 2 * C], F8, tag=f"wqk{g}", name=f"wqk{g}") for g in range(2)]
        wv = [consts.tile([128, 2, C], F8, tag=f"wv{g}", name=f"wv{g}") for g in range(2)]
        wvr = [consts.tile([128, 2, C], F8, tag=f"wvr{g}", name=f"wvr{g}") for g in range(2)]
        for g in range(2):
            nc.sync.dma_start(out=wqk[g], in_=wqk8[g])
            nc.sync.dma_start(out=wv[g], in_=wv8[g])
            nc.sync.dma_start(out=wvr[g], in_=wvr8[g])

        # bias broadcast to all partitions via a K=1 ones-matmul
        bias_ps = ps.tile([128, C], F32, tag="ps")
        nc.tensor.matmul(bias_ps, ones_f32, bias_row, start=True, stop=True)
        bias_bc = consts.tile([128, C], F32)
        nc.vector.tensor_copy(out=bias_bc, in_=bias_ps)

        proj_wT = [consts.tile([128, C], BF16, tag=f"projwT{i}", name=f"projwT{i}") for i in range(4)]

        def emit_proj_w_loads():
            for kc in range(4):
                nc.sync.dma_start(out=proj_wT[kc], in_=proj_wt[ts(kc, 128), :])

        def phase_a(b, interleave, xpre=None):
            """Stream one batch through qkv projection + Gram accumulation.
            ``interleave[i]`` (previous batch's phase-B scalar stages) is
            emitted after chunk i so its ACT/DVE work hides under PE time."""
            if xpre is None:
                xq, xr = make_x_tiles()
                emit_x_granule(b, xq, xr, 0)
            else:
                xq, xr = xpre
            for j in range(1, N // 512):
                emit_x_granule(b, xq, xr, j)
            vT = [vtp.tile([128, N], BF16, tag=f"vt{g}", name=f"vt{g}") for g in range(4)]

            # Gram accumulator: acc2[:, h, :] = (q_h|k_h)^T (q_h|k_h) summed
            # over all token pairs. 2 PSUM banks; per-bank start/stop.
            acc2 = accp.tile([128, H, 128], F32, tag="acc")
            qks_tiles = {}

            def emit_gram(p):
                qks = qks_tiles.pop(p)
                for h in range(H):
                    nc.tensor.matmul(
                        acc2[:, h, :],
                        qks[:, :, h],
                        qks[:, :, h],
                        start=(p == 0 and h % 4 == 0),
                        stop=(p == NPAIR - 1 and h % 4 == 3),
                        perf_mode=DR,
                    )

            for ci in range(NCHUNK):
                t = ci % 2
                # q,k projection (fp8 DoubleRow, stationary = x chunk)
                qp = ps.tile([128, C], F32, tag="ps")
                kp = ps.tile([128, C], F32, tag="ps")
                for g in range(2):
                    nc.tensor.matmul(
                        qp, xq[g][:, :, ts(ci, 128)], wqk[g][:, :, 0:C],
                        start=(g == 0), stop=(g == 1), perf_mode=DR,
                    )
                for g in range(2):
                    nc.tensor.matmul(
                        kp, xq[g][:, :, ts(ci, 128)], wqk[g][:, :, C : 2 * C],
                        start=(g == 0), stop=(g == 1), perf_mode=DR,
                    )
                # fp8 (q_h | k_h) packing, DoubleRow token-tiled by chunk pair
                if t == 0:
                    qks_tiles[ci // 2] = qksp.tile(
                        [128, 2, H, 2, 64], F8, tag="qks", name=f"qks{ci // 2}"
                    )
                qks = qks_tiles[ci // 2]
                nc.scalar.copy(
                    out=qks[:, t, :, 0, :], in_=qp.rearrange("p (h d) -> p h d", h=H)
                )
                nc.vector.tensor_copy(
                    out=qks[:, t, :, 1, :], in_=kp.rearrange("p (h d) -> p h d", h=H)
                )

                # v projection: 3-term residual-corrected fp8 DoubleRow
                nj, mc = ci // 4, ci % 4
                vps = ps.tile([128, 512], F32, tag="ps")
                terms = [(wv, xq), (wv, xr), (wvr, xq)]
                for ti, (wt, xt) in enumerate(terms):
                    for g in range(2):
                        nc.tensor.matmul(
                            vps,
                            wt[g][:, :, ts(mc, 128)],
                            xt[g][:, :, ts(nj, 512)],
                            start=(ti == 0 and g == 0),
                            stop=(ti == 2 and g == 1),
                            perf_mode=DR,
                        )
                # descale to true v while extracting
                nc.scalar.mul(out=vT[mc][:, ts(nj, 512)], in_=vps, mul=1.0 / (SX * SWV))

                # Gram for the previous chunk pair (its copies are done by now)
                if t == 0 and ci >= 2:
                    emit_gram(ci // 2 - 1)
                if ci < len(interleave):
                    interleave[ci]()
            emit_gram(NPAIR - 1)

            # extract logits + masked diagonals immediately so the next batch
            # can reuse the accumulator banks
            attn = small.tile([64, H, 64], F32, tag="attn")
            nc.scalar.copy(out=attn, in_=acc2[0:64, :, 64:128])
            masked = small.tile([128, H, 64], BF16, tag="masked")
            iq = ident_f[0:64, 0:64]
            iq_bc = bass.AP(
                tensor=iq.tensor, offset=iq.offset,
                ap=[list(iq.ap[0]), [0, H], list(iq.ap[1])],
            )
            nc.vector.tensor_tensor(
                out=masked[0:64], in0=acc2[0:64, :, 0:64], in1=iq_bc,
                op=mybir.AluOpType.mult,
            )
            ik = ident_f[64:128, 64:128]
            ik_bc = bass.AP(
                tensor=ik.tensor, offset=ik.offset,
                ap=[list(ik.ap[0]), [0, H], list(ik.ap[1])],
            )
            nc.vector.tensor_tensor(
                out=masked[64:128], in0=acc2[64:128, :, 64:128], in1=ik_bc,
                op=mybir.AluOpType.mult,
            )
            return attn, masked, vT

        def phase_b_stages(b, attn, masked, vT):
            """Returns ([s1..s5] scalar stages to interleave elsewhere, and
            final_block(interleave2) with the attn@v + proj matmuls)."""
            ssqd = small.tile([64, H], F32, tag="ssqd")
            invq = small.tile([64, H], F32, tag="invq")
            krow = small.tile([1, H, 64], F32, tag="krow")
            krow_b = small.tile([1, H, 64], BF16, tag="krowb")
            ikb = small.tile([64, H, 64], F32, tag="ikb")
            probs = small.tile([64, H, 64], BF16, tag="probs")
            bd = small.tile([128, 4, 128], BF16, tag="bd")
            mx = small.tile([64, H], F32, tag="mx")
            ex = small.tile([64, H, 64], F32, tag="ex")
            rs = small.tile([64, H], F32, tag="rs")

            def s1():
                # inv_q = 1/max(sqrt(ssq_q), eps) per (d, h), partition-major
                nc.vector.tensor_reduce(
                    out=ssqd, in_=masked[0:64], axis=mybir.AxisListType.X,
                    op=mybir.AluOpType.add,
                )
                nc.scalar.sqrt(out=invq, in_=ssqd)
                nc.vector.tensor_scalar_max(invq, invq, EPS)
                nc.vector.reciprocal(out=invq, in_=invq)
                # ssq_k as a [1, H*64] row: ones^T @ masked k-half
                krow_ps = ps.tile([1, H, 64], F32, tag="ps")
                nc.tensor.matmul(
                    krow_ps, ones_col[64:128, :], masked[64:128],
                    start=True, stop=True,
                )
                nc.vector.tensor_copy(out=krow, in_=krow_ps)

            def s2():
                # inv_k row, temperature folded in (uniform over d AND e per h)
                nc.scalar.sqrt(out=krow, in_=krow)
                nc.vector.tensor_scalar_max(krow, krow, EPS)
                nc.vector.reciprocal(out=krow, in_=krow)
                temp_bc = bass.AP(
                    tensor=temp_sb.tensor, offset=temp_sb.offset,
                    ap=[list(temp_sb.ap[0]), [1, H], [0, D]],
                )
                nc.vector.tensor_tensor(
                    out=krow_b, in0=krow, in1=temp_bc, op=mybir.AluOpType.mult
                )

            def s3():
                # broadcast inv_k*temp over the 64 d-partitions via K=1 matmul
                ikb_ps = ps.tile([64, H, 64], F32, tag="ps")
                nc.tensor.matmul(
                    ikb_ps, ones_row[:, 0:64],
                    krow_b.rearrange("p h d -> p (h d)"),
                    start=True, stop=True,
                )
                nc.vector.tensor_copy(out=ikb, in_=ikb_ps)
                nc.vector.tensor_mul(out=attn, in0=attn, in1=ikb)
                invq_bc = bass.AP(
                    tensor=invq.tensor, offset=invq.offset,
                    ap=[list(invq.ap[0]), list(invq.ap[1]), [0, D]],
                )
                nc.vector.tensor_tensor(
                    out=attn, in0=attn, in1=invq_bc, op=mybir.AluOpType.mult
                )

            def s4():
                # softmax over the last axis (per head)
                nc.vector.tensor_reduce(
                    out=mx, in_=attn, axis=mybir.AxisListType.X,
                    op=mybir.AluOpType.max, negate=True,
                )
                mx_bc = bass.AP(
                    tensor=mx.tensor, offset=mx.offset,
                    ap=[list(mx.ap[0]), list(mx.ap[1]), [0, D]],
                )
                nc.vector.tensor_tensor(
                    out=attn, in0=attn, in1=mx_bc, op=mybir.AluOpType.add
                )
                nc.scalar.activation(
                    out=ex, in_=attn, func=mybir.ActivationFunctionType.Exp
                )
                nc.vector.tensor_reduce(
                    out=rs, in_=ex, axis=mybir.AxisListType.X,
                    op=mybir.AluOpType.add,
                )
                nc.vector.reciprocal(out=rs, in_=rs)
                rs_bc = bass.AP(
                    tensor=rs.tensor, offset=rs.offset,
                    ap=[list(rs.ap[0]), list(rs.ap[1]), [0, D]],
                )
                nc.vector.tensor_tensor(
                    out=probs, in0=ex, in1=rs_bc, op=mybir.AluOpType.mult
                )

            def s5():
                # block-diagonal packed attn^T: pair g holds head 2g at
                # [0:64, 0:64] and head 2g+1 at [64:128, 64:128]
                atp = ps.tile([128, 4, 64], BF16, tag="ps")
                for h in range(H):
                    r, g = h % 2, h // 2
                    nc.tensor.transpose(
                        atp[r * 64 : r * 64 + 64, g, :],
                        probs[:, h, :],
                        ident_b[0:64, 0:64],
                    )
                nc.vector.memset(bd, 0.0)
                nc.scalar.copy(out=bd[0:64, :, 0:64], in_=atp[0:64])
                nc.scalar.copy(out=bd[64:128, :, 64:128], in_=atp[64:128])

            def final_block(interleave2):
                outTs = {}

                def av(nj):
                    outT = [
                        outp.tile([128, 512], BF16, tag=f"outT{g}", name=f"outT{g}")
                        for g in range(4)
                    ]
                    for g in range(4):
                        ops = ps.tile([128, 512], F32, tag="ps")
                        nc.tensor.matmul(
                            ops, bd[:, g, :], vT[g][:, ts(nj, 512)],
                            start=True, stop=True,
                        )
                        # split extraction across ACT and DVE
                        if g % 2 == 0:
                            nc.scalar.copy(out=outT[g], in_=ops)
                        else:
                            nc.vector.tensor_copy(out=outT[g], in_=ops)
                    outTs[nj] = outT

                def pj(nj):
                    outT = outTs.pop(nj)
                    for t4 in range(4):
                        ypt = ps.tile([128, C], F32, tag="ps")
                        for kc in range(4):
                            nc.tensor.matmul(
                                ypt, outT[kc][:, ts(t4, 128)], proj_wT[kc],
                                start=(kc == 0), stop=(kc == 3),
                            )
                        ysb = yp.tile([128, C], F32, tag="ysb")
                        nc.vector.tensor_add(out=ysb, in0=ypt, in1=bias_bc)
                        nc.sync.dma_start(
                            out=out[b, nj * 512 + t4 * 128 : nj * 512 + (t4 + 1) * 128, :],
                            in_=ysb,
                        )

                av(0)
                av(1)
                for nj in range(N // 512):
                    if nj + 2 < N // 512:
                        av(nj + 2)
                    pj(nj)
                    if nj < len(interleave2):
                        interleave2[nj]()

            return [s1, s2, s3, s4, s5], final_block

        prev = None
        for b in range(BL):
            st = prev[0] if prev else []
            ctx = phase_a(b, st, xpre=x0 if b == 0 else None)
            if b == 0:
                emit_proj_w_loads()
            newp = phase_b_stages(b, *ctx)
            if prev is not None:
                # previous batch's matmul block; on the last batch also hide
                # the current batch's scalar chain inside it
                prev[1](newp[0] if b == BL - 1 else [])
                if b == BL - 1:
                    newp = (newp[0][len(newp[0]) :], newp[1])  # stages consumed
            prev = newp
        for s in prev[0]:
            s()
        prev[1]([])

        accp.release()
        ps.release()
        yp.release()
        outp.release()
        small.release()
        qksp.release()
        vtp.release()
        xtp.release()
        consts.release()

    legalize_waits(nc)
    return nc


def build_trivial_bass():
    """Minimal kernel used by the benchmark harness to measure the
    per-dispatch floor (axon round trip + runtime overhead)."""
    nc = bass.Bass(trn_type="TRN2")
    inp = nc.dram_tensor("inp", [128, 512], F32, kind="ExternalInput")
    outp = nc.dram_tensor("outp", [128, 512], F32, kind="ExternalOutput")
    with TileContext(nc) as tc:
        with tc.tile_pool(name="p", bufs=1) as pool:
            s = pool.tile([128, 512], F32)
            nc.sync.dma_start(out=s, in_=inp[:, :])
            nc.sync.dma_start(out=outp[:, :], in_=s)
    legalize_waits(nc)
    return nc


_NC_CACHE = {}


def _pack_rows(a):
    """[C, cols] -> [kc2, k, t, cols] with row c = kc2*256 + t*128 + k."""
    Crows, cols = a.shape
    return np.ascontiguousarray(
        a.reshape(2, 2, 128, cols).transpose(0, 2, 1, 3)
    )


def make_in_maps(x, qkv_w, temperature, proj_w, proj_b):
    import ml_dtypes

    f8 = ml_dtypes.float8_e4m3  # matches device float8e4 (IEEE e4m3)
    bf = ml_dtypes.bfloat16
    x = np.asarray(x, np.float32)
    qkv_wt = np.asarray(qkv_w, np.float32).T  # [C, 3C]
    temp_flat = np.ascontiguousarray(np.asarray(temperature, np.float32).reshape(H))
    proj_wt = np.ascontiguousarray(np.asarray(proj_w, np.float32).T.astype(bf))
    pb = np.ascontiguousarray(np.asarray(proj_b, np.float32))

    wqk8 = _pack_rows(SWQ * qkv_wt[:, 0 : 2 * C]).astype(f8)
    wv_s = SWV * qkv_wt[:, 2 * C :]
    wv8 = wv_s.astype(f8)
    wvr8 = _pack_rows(wv_s - wv8.astype(np.float32)).astype(f8)
    wv8 = _pack_rows(wv8.astype(np.float32)).astype(f8)

    in_maps = []
    for i in range(NCORES):
        xs = SX * x[i * BL : (i + 1) * BL].transpose(0, 2, 1)  # [BL, C, N]
        x8 = xs.astype(f8)
        xr8f = xs - x8.astype(np.float32)
        xq8 = np.stack([_pack_rows(x8[bb].astype(np.float32)) for bb in range(BL)]).astype(f8)
        xr8 = np.stack([_pack_rows(xr8f[bb]) for bb in range(BL)]).astype(f8)
        in_maps.append(
            {
                "xq8": xq8,
                "xr8": xr8,
                "wqk8": wqk8,
                "wv8": wv8,
                "wvr8": wvr8,
                "temperature": temp_flat,
                "proj_wt": proj_wt,
                "proj_b": pb,
            }
        )
    return in_maps


def kernel(x, qkv_w, temperature, proj_w, proj_b, _want_trace=False, _trace_kwargs=None):
    key = MM_MODE
    if key not in _NC_CACHE:
        _NC_CACHE[key] = build_bass()
    nc = _NC_CACHE[key]

    in_maps = make_in_maps(x, qkv_w, temperature, proj_w, proj_b)
    res = run_bass_kernel_spmd(
        nc,
        in_maps,
        core_ids=list(range(NCORES)),
        trace=_want_trace,
        **(_trace_kwargs or {}),
    )
    y = np.concatenate([res.results[i]["out"] for i in range(NCORES)], axis=0)
    if _want_trace:
        return y, res
    return y


# revision 9
# speedup vs baseline: 2.2761x; 1.0198x over previous
"""Trainium2 Bass kernel for nn_CABlock (channel attention / XCA block).

Reference computation (per batch b):
  qkv = x @ qkv_w.T                      # [N, 3C], token-major
  q,k,v per head: [d=64, N] channel-major after reshape/transpose
  q,k l2-normalized over N; attn = softmax((q @ k.T) * temperature, axis=-1)
  out = attn @ v  -> [N, C];  y = out @ proj_w.T + proj_b

Numerics / restructure:
  * l2norm commutes with the bilinear form:
      logits = diag(inv_q) @ (q_raw @ k_raw.T) @ diag(inv_k) * temp
    so any uniform scaling of q/k (from fp8 pre-scaling) cancels exactly.
  * q/k projection and the token Gram run in fp8 (e4m3) DoubleRow mode:
    256-deep contraction at 0.5 PE cycles per output column, 4x bf16
    throughput.  Softmax over the tiny normalized logits washes out the
    quantization noise (validated ~4.5e-3 rel err end to end).  Device
    float8e4 is IEEE e4m3 (max 240, overflow -> inf, no saturation), so
    power-of-two pre-scales keep all fp8 values under ~100.
  * v projection is a 3-term residual-corrected fp8 product
      v = [W8@x8 + W8@xr8 + Wr8@x8] / (SX*SWV)
    with x8=fp8(SX*x), xr8=fp8(SX*x-x8), W8=fp8(SWV*W), Wr8=fp8(SWV*W-W8)
    all host-prepared; the dropped Wr*xr term is second order.
  * per head ONE Gram matmul with stationary=moving=(q_h|k_h) packed along
    the free axis (k|q for odd heads): the cross block is the logits and
    the diagonals of the self blocks are the q/k norms -- free.  The
    parity swap places odd heads' logits on partitions 64-127 so ALL
    later phase-B tensors live on 128 partitions, partition-aligned.
  * attn@v + projection are fused associatively:
      y = v_cm^T @ (blockdiag(probs) @ proj_w^T)
    The tiny per-batch G = BD@pwT costs 4 matmuls of 512 columns instead
    of mixing attention into the N=4096-wide v, eliminating the whole
    attn@v stage and its PSUM-extraction copies.
  * x arrives host-transposed channel-major (DoubleRow k-tile packed); no
    on-device transposes anywhere.  The phase-B scalar chain of batch b
    is interleaved into the following batch's phase A (or the previous
    batch's projection loop for the last batch) so the PE never waits.

Sharding: data-parallel over batch B=16 across 8 cores (2 batches/core).
No collectives needed.
"""

import os
import sys

import numpy as np

for _p in ("/opt/trn_rl_repo", "/root/.axon_site/_ro/trn_rl_repo"):
    if os.path.isdir(_p) and _p not in sys.path:
        sys.path.insert(0, _p)

import concourse.bass as bass  # noqa: E402
from concourse import mybir  # noqa: E402
from concourse.bass import ts  # noqa: E402
from concourse.bass_utils import run_bass_kernel_spmd  # noqa: E402
from concourse.masks import make_identity  # noqa: E402
from concourse.tile import TileContext  # noqa: E402

B, N, C = 16, 4096, 512
H, D = 8, 64
C3 = 3 * C
NCORES = 8
BL = B // NCORES  # batches per core
EPS = 1e-12
NCHUNK = N // 128  # 32 token chunks per batch
NPAIR = NCHUNK // 2
F32 = mybir.dt.float32
BF16 = mybir.dt.bfloat16
F8 = mybir.dt.float8e4
DR = mybir.MatmulPerfMode.DoubleRow

# fp8 pre-scales (powers of two; exactly cancelled on-device). Device
# float8e4 is IEEE e4m3: max finite 240, NO saturation (overflow -> inf),
# so scales keep every fp8 value comfortably under ~100.
SX = 2.0  # x  (|2x| <~ 11)
SWQ = 16.0  # q,k weight columns (|32*q| <~ 80 for the fp8 qks re-quant)
SWV = 256.0  # v weight columns (|256*wv| <~ 26)

MM_MODE = "bf16"  # kept for test.py compatibility


def legalize_waits(nc):
    """Walrus in this environment rejects instructions carrying more than one
    semaphore wait ("Too many sync wait commands"), and rejects sem-ge waits
    on Drain instructions entirely. Tile emits both. Hoist the offending
    waits onto standalone EventSemaphore instructions inserted immediately
    before the instruction on the same engine queue — semantically identical
    (the engine executes the waits, then the instruction)."""
    n_new = 0
    for bb in nc.main_func.blocks:
        il = bb.instructions
        new_list = []
        for ins in il:
            si = ins.sync_info
            waits = list(si.on_wait) if si is not None and si.on_wait else []
            if waits:
                tname = type(ins).__name__
                no_wait_slots = tname in ("InstDrain", "InstDmaTransposeAnt") or (
                    getattr(ins, "opcode", "") in ("Drain", "DmaTransposeAnt")
                )
                keep_budget = 0 if no_wait_slots else 1
                if len(waits) > keep_budget:
                    hoist, keep = waits[:-keep_budget] if keep_budget else waits, (
                        waits[-keep_budget:] if keep_budget else []
                    )
                    for w in hoist:
                        ev = mybir.InstEventSemaphore(
                            name=f"{ins.name}-hoistw{n_new}",
                            ins=[],
                            outs=[],
                            engine=ins.engine,
                            sync_info=mybir.SyncInfo(on_wait=[w], on_update=[]),
                        )
                        new_list.append(ev)
                        n_new += 1
                    ins.sync_info = mybir.SyncInfo(
                        on_wait=keep, on_update=list(si.on_update or [])
                    )
            new_list.append(ins)
        il.clear()
        il.extend(new_list)
    return n_new


def build_bass():
    nc = bass.Bass(trn_type="TRN2")
    # channel-major, DoubleRow k-tile packed: [b, kc2, k, t, n] = channel
    # kc2*256 + t*128 + k of batch b (value pre-scaled by SX, fp8)
    xq8 = nc.dram_tensor("xq8", [BL, 2, 128, 2, N], F8, kind="ExternalInput")
    xr8 = nc.dram_tensor("xr8", [BL, 2, 128, 2, N], F8, kind="ExternalInput")
    # q,k weight cols (x SWQ): [kc2, k, t, j] = qkv_wt[kc2*256+t*128+k, j]
    wqk8 = nc.dram_tensor("wqk8", [2, 128, 2, 2 * C], F8, kind="ExternalInput")
    wv8 = nc.dram_tensor("wv8", [2, 128, 2, C], F8, kind="ExternalInput")
    wvr8 = nc.dram_tensor("wvr8", [2, 128, 2, C], F8, kind="ExternalInput")
    # temperature pre-arranged [par, j]: par 0 = odd heads, par 1 = even
    temp = nc.dram_tensor("temperature", [2, 4], F32, kind="ExternalInput")
    proj_wt = nc.dram_tensor("proj_wt", [C, C], BF16, kind="ExternalInput")
    proj_b = nc.dram_tensor("proj_b", [C], F32, kind="ExternalInput")
    out = nc.dram_tensor("out", [BL, N, C], F32, kind="ExternalOutput")

    with TileContext(nc) as tc:
        consts = tc.alloc_tile_pool(name="consts", bufs=1)
        xtp = tc.alloc_tile_pool(name="xtp", bufs=2)
        vtp = tc.alloc_tile_pool(name="vtp", bufs=2)
        qksp = tc.alloc_tile_pool(name="qksp", bufs=4)
        small = tc.alloc_tile_pool(name="small", bufs=2)
        yp = tc.alloc_tile_pool(name="yp", bufs=3)
        ps = tc.alloc_tile_pool(name="ps", bufs=6, space="PSUM")
        accp = tc.alloc_tile_pool(name="accp", bufs=1, space="PSUM")

        # ---- tiny constants (cheap DMAs first) ----
        temp_sb = consts.tile([1, 2, 4], F32)
        nc.sync.dma_start(out=temp_sb, in_=temp[:])
        bias_row = consts.tile([1, C], F32)
        nc.sync.dma_start(out=bias_row, in_=proj_b[:])
        ident_f = consts.tile([128, 128], F32)
        make_identity(nc, ident_f)
        ones_col = consts.tile([128, 1], BF16)
        nc.vector.memset(ones_col, 1.0)
        ones_row = consts.tile([1, 128], BF16)
        nc.vector.memset(ones_row, 1.0)
        ones_f32 = consts.tile([1, 128], F32)
        nc.vector.memset(ones_f32, 1.0)

        # ---- batch-0 x granule j=0 before the bulk weight loads so chunk 0
        # can start as early as possible ----
        def make_x_tiles():
            xq = [xtp.tile([128, 2, N], F8, tag=f"xq{g}", name=f"xq{g}") for g in range(2)]
            xr = [xtp.tile([128, 2, N], F8, tag=f"xr{g}", name=f"xr{g}") for g in range(2)]
            return xq, xr

        def emit_x_granule(b, xq, xr, j):
            for g in range(2):
                nc.sync.dma_start(out=xq[g][:, :, ts(j, 512)], in_=xq8[b, g, :, :, ts(j, 512)])
                nc.sync.dma_start(out=xr[g][:, :, ts(j, 512)], in_=xr8[b, g, :, :, ts(j, 512)])

        x0 = make_x_tiles()
        emit_x_granule(0, x0[0], x0[1], 0)

        # ---- weights needed by phase A ----
        wqk = [consts.tile([128, 2, 2 * C], F8, tag=f"wqk{g}", name=f"wqk{g}") for g in range(2)]
        wv = [consts.tile([128, 2, C], F8, tag=f"wv{g}", name=f"wv{g}") for g in range(2)]
        wvr = [consts.tile([128, 2, C], F8, tag=f"wvr{g}", name=f"wvr{g}") for g in range(2)]
        for g in range(2):
            nc.sync.dma_start(out=wqk[g], in_=wqk8[g])
            nc.sync.dma_start(out=wv[g], in_=wv8[g])
            nc.sync.dma_start(out=wvr[g], in_=wvr8[g])

        proj_wT = [consts.tile([128, C], BF16, tag=f"projwT{i}", name=f"projwT{i}") for i in range(4)]
        bias_bc = consts.tile([128, C], F32)

        def emit_deferred_consts():
            # proj weights + bias broadcast: first needed ~100us in
            for kc in range(4):
                nc.sync.dma_start(out=proj_wT[kc], in_=proj_wt[ts(kc, 128), :])
            bias_ps = ps.tile([128, C], F32, tag="ps")
            nc.tensor.matmul(bias_ps, ones_f32, bias_row, start=True, stop=True)
            nc.vector.tensor_copy(out=bias_bc, in_=bias_ps)

        def phase_a(b, interleave, xpre=None):
            """Stream one batch through qkv projection + Gram accumulation.
            ``interleave[i]`` (previous batch's phase-B scalar stages) is
            emitted after chunk i so its ACT/DVE work hides under PE time."""
            if xpre is None:
                xq, xr = make_x_tiles()
                emit_x_granule(b, xq, xr, 0)
            else:
                xq, xr = xpre
            for j in range(1, N // 512):
                emit_x_granule(b, xq, xr, j)
            vT = [vtp.tile([128, N], BF16, tag=f"vt{g}", name=f"vt{g}") for g in range(4)]

            # Gram accumulator: acc2[:, h, :] = s_h^T s_h with s_h=(q_h|k_h)
            # for even h, (k_h|q_h) for odd h.  2 PSUM banks.
            acc2 = accp.tile([128, H, 128], F32, tag="acc")
            qks_tiles = {}

            def emit_gram(p):
                qks = qks_tiles.pop(p)
                for h in range(H):
                    nc.tensor.matmul(
                        acc2[:, h, :],
                        qks[:, :, h],
                        qks[:, :, h],
                        start=(p == 0 and h % 4 == 0),
                        stop=(p == NPAIR - 1 and h % 4 == 3),
                        perf_mode=DR,
                    )

            for ci in range(NCHUNK):
                t = ci % 2
                # q,k projection (fp8 DoubleRow, stationary = x chunk)
                qp = ps.tile([128, C], F32, tag="ps")
                kp = ps.tile([128, C], F32, tag="ps")
                for g in range(2):
                    nc.tensor.matmul(
                        qp, xq[g][:, :, ts(ci, 128)], wqk[g][:, :, 0:C],
                        start=(g == 0), stop=(g == 1), perf_mode=DR,
                    )
                for g in range(2):
                    nc.tensor.matmul(
                        kp, xq[g][:, :, ts(ci, 128)], wqk[g][:, :, C : 2 * C],
                        start=(g == 0), stop=(g == 1), perf_mode=DR,
                    )
                # fp8 (q_h|k_h) / (k_h|q_h) packing (parity-swapped so odd
                # heads' logits land on partitions 64-127)
                if t == 0:
                    qks_tiles[ci // 2] = qksp.tile(
                        [128, 2, H, 2, 64], F8, tag="qks", name=f"qks{ci // 2}"
                    )
                qks = qks_tiles[ci // 2]
                qpr = qp.rearrange("p (h d) -> p h d", h=H)
                kpr = kp.rearrange("p (h d) -> p h d", h=H)
                nc.scalar.copy(out=qks[:, t, 0::2, 0, :], in_=qpr[:, 0::2, :])
                nc.scalar.copy(out=qks[:, t, 1::2, 1, :], in_=qpr[:, 1::2, :])
                nc.vector.tensor_copy(out=qks[:, t, 0::2, 1, :], in_=kpr[:, 0::2, :])
                nc.vector.tensor_copy(out=qks[:, t, 1::2, 0, :], in_=kpr[:, 1::2, :])

                # v projection: 3-term residual-corrected fp8 DoubleRow
                nj, mc = ci // 4, ci % 4
                vps = ps.tile([128, 512], F32, tag="ps")
                terms = [(wv, xq), (wv, xr), (wvr, xq)]
                for ti, (wt, xt) in enumerate(terms):
                    for g in range(2):
                        nc.tensor.matmul(
                            vps,
                            wt[g][:, :, ts(mc, 128)],
                            xt[g][:, :, ts(nj, 512)],
                            start=(ti == 0 and g == 0),
                            stop=(ti == 2 and g == 1),
                            perf_mode=DR,
                        )
                # descale to true v while extracting; alternate engines to
                # keep both ACT and DVE under the PE chunk rate
                if ci % 2 == 0:
                    nc.scalar.mul(out=vT[mc][:, ts(nj, 512)], in_=vps, mul=1.0 / (SX * SWV))
                else:
                    nc.vector.tensor_scalar_mul(
                        vT[mc][:, ts(nj, 512)], vps, 1.0 / (SX * SWV)
                    )

                # Gram for the previous chunk pair (its copies are done)
                if t == 0 and ci >= 2:
                    emit_gram(ci // 2 - 1)
                if ci < len(interleave):
                    interleave[ci]()
            emit_gram(NPAIR - 1)

            # extract logits + masked diagonals immediately so the next batch
            # can reuse the accumulator banks.  Layout notes (j = h//2):
            #   even h: q^T k at [0:64, 64:128], qq at [0:64,0:64], kk at
            #           [64:128,64:128]
            #   odd h:  q^T k (as [dq,e]) at [64:128, 0:64], kk at
            #           [0:64,0:64], qq at [64:128,64:128]
            attn = small.tile([128, 4, 64], F32, tag="attn")
            nc.scalar.copy(out=attn[0:64], in_=acc2[0:64, 0::2, 64:128])
            nc.scalar.copy(out=attn[64:128], in_=acc2[64:128, 1::2, 0:64])
            maskq = small.tile([128, 4, 64], BF16, tag="maskq")
            maskk = small.tile([128, 4, 64], BF16, tag="maskk")
            iq = ident_f[0:64, 0:64]
            iq_bc = bass.AP(
                tensor=iq.tensor, offset=iq.offset,
                ap=[list(iq.ap[0]), [0, 4], list(iq.ap[1])],
            )
            ik = ident_f[64:128, 64:128]
            ik_bc = bass.AP(
                tensor=ik.tensor, offset=ik.offset,
                ap=[list(ik.ap[0]), [0, 4], list(ik.ap[1])],
            )
            mul = mybir.AluOpType.mult
            nc.vector.tensor_tensor(out=maskq[0:64], in0=acc2[0:64, 0::2, 0:64], in1=iq_bc, op=mul)
            nc.vector.tensor_tensor(out=maskq[64:128], in0=acc2[64:128, 1::2, 64:128], in1=ik_bc, op=mul)
            nc.vector.tensor_tensor(out=maskk[0:64], in0=acc2[0:64, 1::2, 0:64], in1=iq_bc, op=mul)
            nc.vector.tensor_tensor(out=maskk[64:128], in0=acc2[64:128, 0::2, 64:128], in1=ik_bc, op=mul)
            return attn, maskq, maskk, vT

        def phase_b_stages(b, attn, maskq, maskk, vT):
            """Returns ([s...] scalar stages to interleave elsewhere, and
            final_block(interleave2) = the fused projection loop)."""
            ssqd = small.tile([128, 4], F32, tag="ssqd")
            invq = small.tile([128, 4], F32, tag="invq")
            # kr[par]: par 0 = odd heads (from maskk top), 1 = even (bottom)
            kr = small.tile([1, 2, 4, 64], F32, tag="kr")
            kr_b = small.tile([1, 2, 4, 64], BF16, tag="krb")
            ikb = small.tile([128, 4, 64], F32, tag="ikb")
            probs = small.tile([128, 4, 64], BF16, tag="probs")
            bd = small.tile([128, 4, 128], BF16, tag="bd")
            gsb = small.tile([128, 4, C], BF16, tag="gsb")
            mx = small.tile([128, 4], F32, tag="mx")
            ex = small.tile([128, 4, 64], F32, tag="ex")
            rs = small.tile([128, 4], F32, tag="rs")

            def s1():
                # inv_q = 1/max(sqrt(ssq_q), eps) per (d, h), partition-major
                nc.vector.tensor_reduce(
                    out=ssqd, in_=maskq, axis=mybir.AxisListType.X,
                    op=mybir.AluOpType.add,
                )
                nc.scalar.sqrt(out=invq, in_=ssqd)
                nc.vector.tensor_scalar_max(invq, invq, EPS)
                nc.vector.reciprocal(out=invq, in_=invq)
                # ssq_k rows via ones^T @ masked k diags (one per parity)
                kr_ps = ps.tile([1, 2, 4, 64], F32, tag="ps")
                nc.tensor.matmul(kr_ps[:, 0], ones_col[0:64, :], maskk[0:64], start=True, stop=True)
                nc.tensor.matmul(kr_ps[:, 1], ones_col[64:128, :], maskk[64:128], start=True, stop=True)
                nc.vector.tensor_copy(out=kr, in_=kr_ps)

            def s2():
                # inv_k rows, temperature folded in (uniform over d AND e)
                nc.scalar.sqrt(out=kr, in_=kr)
                nc.vector.tensor_scalar_max(kr, kr, EPS)
                nc.vector.reciprocal(out=kr, in_=kr)
                temp_bc = bass.AP(
                    tensor=temp_sb.tensor, offset=temp_sb.offset,
                    ap=[list(temp_sb.ap[0]), [4, 2], [1, 4], [0, D]],
                )
                nc.vector.tensor_tensor(
                    out=kr_b, in0=kr, in1=temp_bc, op=mybir.AluOpType.mult
                )

            def s3():
                # broadcast inv_k*temp over d-partitions: top half needs even
                # heads (par 1), bottom half odd heads (par 0)
                ikb_ps = ps.tile([128, 4, 64], F32, tag="ps")
                nc.tensor.matmul(
                    ikb_ps[0:64], ones_row[:, 0:64],
                    kr_b[:, 1].rearrange("p h d -> p (h d)"),
                    start=True, stop=True,
                )
                nc.tensor.matmul(
                    ikb_ps[64:128], ones_row[:, 0:64],
                    kr_b[:, 0].rearrange("p h d -> p (h d)"),
                    start=True, stop=True,
                )
                nc.vector.tensor_copy(out=ikb, in_=ikb_ps)
                nc.vector.tensor_mul(out=attn, in0=attn, in1=ikb)
                invq_bc = bass.AP(
                    tensor=invq.tensor, offset=invq.offset,
                    ap=[list(invq.ap[0]), list(invq.ap[1]), [0, D]],
                )
                nc.vector.tensor_tensor(
                    out=attn, in0=attn, in1=invq_bc, op=mybir.AluOpType.mult
                )

            def s4():
                # softmax over the last axis (per head)
                nc.vector.tensor_reduce(
                    out=mx, in_=attn, axis=mybir.AxisListType.X,
                    op=mybir.AluOpType.max, negate=True,
                )
                mx_bc = bass.AP(
                    tensor=mx.tensor, offset=mx.offset,
                    ap=[list(mx.ap[0]), list(mx.ap[1]), [0, D]],
                )
                nc.vector.tensor_tensor(
                    out=attn, in0=attn, in1=mx_bc, op=mybir.AluOpType.add
                )
                nc.scalar.activation(
                    out=ex, in_=attn, func=mybir.ActivationFunctionType.Exp
                )
                nc.vector.tensor_reduce(
                    out=rs, in_=ex, axis=mybir.AxisListType.X,
                    op=mybir.AluOpType.add,
                )
                nc.vector.reciprocal(out=rs, in_=rs)
                rs_bc = bass.AP(
                    tensor=rs.tensor, offset=rs.offset,
                    ap=[list(rs.ap[0]), list(rs.ap[1]), [0, D]],
                )
                nc.vector.tensor_tensor(
                    out=probs, in0=ex, in1=rs_bc, op=mybir.AluOpType.mult
                )

            def s5():
                # blockdiag(probs) [dq, e] per pair: even head at [0:64,0:64],
                # odd head at [64:128,64:128] -- partition-aligned copies
                nc.vector.memset(bd, 0.0)
                nc.scalar.copy(out=bd[0:64, :, 0:64], in_=probs[0:64])
                nc.scalar.copy(out=bd[64:128, :, 64:128], in_=probs[64:128])

            def make_g(g):
                def sg():
                    # G_g = BD_g @ pwT_g : mix attention into the projection
                    g_ps = ps.tile([128, C], F32, tag="ps")
                    nc.tensor.matmul(g_ps, bd[:, g, :], proj_wT[g], start=True, stop=True)
                    if g % 2 == 0:
                        nc.scalar.copy(out=gsb[:, g, :], in_=g_ps)
                    else:
                        nc.vector.tensor_copy(out=gsb[:, g, :], in_=g_ps)
                return sg

            def final_block(interleave2):
                for nj in range(NCHUNK):
                    ypt = ps.tile([128, C], F32, tag="ps")
                    for g in range(4):
                        nc.tensor.matmul(
                            ypt, vT[g][:, ts(nj, 128)], gsb[:, g, :],
                            start=(g == 0), stop=(g == 3),
                        )
                    ysb = yp.tile([128, C], F32, tag="ysb")
                    nc.vector.tensor_add(out=ysb, in0=ypt, in1=bias_bc)
                    nc.sync.dma_start(out=out[b, ts(nj, 128), :], in_=ysb)
                    if nj < len(interleave2):
                        interleave2[nj]()

            stages = [s1, s2, s3, s4, s5] + [make_g(g) for g in range(4)]
            return stages, final_block

        prev = None
        for b in range(BL):
            st = prev[0] if prev else []
            ctx = phase_a(b, st, xpre=x0 if b == 0 else None)
            if b == 0:
                emit_deferred_consts()
            newp = phase_b_stages(b, *ctx)
            if prev is not None:
                # previous batch's projection loop; on the last batch also
                # hide the current batch's scalar chain inside it
                prev[1](newp[0] if b == BL - 1 else [])
                if b == BL - 1:
                    newp = ([], newp[1])
            prev = newp
        for s in prev[0]:
            s()
        prev[1]([])

        accp.release()
        ps.release()
        yp.release()
        small.release()
        qksp.release()
        vtp.release()
        xtp.release()
        consts.release()

    legalize_waits(nc)
    return nc


def build_trivial_bass():
    """Minimal kernel used by the benchmark harness to measure the
    per-dispatch floor (axon round trip + runtime overhead)."""
    nc = bass.Bass(trn_type="TRN2")
    inp = nc.dram_tensor("inp", [128, 512], F32, kind="ExternalInput")
    outp = nc.dram_tensor("outp", [128, 512], F32, kind="ExternalOutput")
    with TileContext(nc) as tc:
        with tc.tile_pool(name="p", bufs=1) as pool:
            s = pool.tile([128, 512], F32)
            nc.sync.dma_start(out=s, in_=inp[:, :])
            nc.sync.dma_start(out=outp[:, :], in_=s)
    legalize_waits(nc)
    return nc


_NC_CACHE = {}


def _pack_rows(a):
    """[C, cols] -> [kc2, k, t, cols] with row c = kc2*256 + t*128 + k."""
    Crows, cols = a.shape
    return np.ascontiguousarray(
        a.reshape(2, 2, 128, cols).transpose(0, 2, 1, 3)
    )


def make_in_maps(x, qkv_w, temperature, proj_w, proj_b):
    import ml_dtypes

    f8 = ml_dtypes.float8_e4m3  # matches device float8e4 (IEEE e4m3)
    bf = ml_dtypes.bfloat16
    x = np.asarray(x, np.float32)
    qkv_wt = np.asarray(qkv_w, np.float32).T  # [C, 3C]
    tf = np.asarray(temperature, np.float32).reshape(H)
    # [par, j]: par 0 = odd heads, par 1 = even heads
    temp_arr = np.ascontiguousarray(np.stack([tf[1::2], tf[0::2]]))
    proj_wt = np.ascontiguousarray(np.asarray(proj_w, np.float32).T.astype(bf))
    pb = np.ascontiguousarray(np.asarray(proj_b, np.float32))

    wqk8 = _pack_rows(SWQ * qkv_wt[:, 0 : 2 * C]).astype(f8)
    wv_s = SWV * qkv_wt[:, 2 * C :]
    wv8 = wv_s.astype(f8)
    wvr8 = _pack_rows(wv_s - wv8.astype(np.float32)).astype(f8)
    wv8 = _pack_rows(wv8.astype(np.float32)).astype(f8)

    in_maps = []
    for i in range(NCORES):
        xs = SX * x[i * BL : (i + 1) * BL].transpose(0, 2, 1)  # [BL, C, N]
        x8 = xs.astype(f8)
        xr8f = xs - x8.astype(np.float32)
        xq8 = np.stack([_pack_rows(x8[bb].astype(np.float32)) for bb in range(BL)]).astype(f8)
        xr8 = np.stack([_pack_rows(xr8f[bb]) for bb in range(BL)]).astype(f8)
        in_maps.append(
            {
                "xq8": xq8,
                "xr8": xr8,
                "wqk8": wqk8,
                "wv8": wv8,
                "wvr8": wvr8,
                "temperature": temp_arr,
                "proj_wt": proj_wt,
                "proj_b": pb,
            }
        )
    return in_maps


def kernel(x, qkv_w, temperature, proj_w, proj_b, _want_trace=False, _trace_kwargs=None):
    key = MM_MODE
    if key not in _NC_CACHE:
        _NC_CACHE[key] = build_bass()
    nc = _NC_CACHE[key]

    in_maps = make_in_maps(x, qkv_w, temperature, proj_w, proj_b)
    res = run_bass_kernel_spmd(
        nc,
        in_maps,
        core_ids=list(range(NCORES)),
        trace=_want_trace,
        **(_trace_kwargs or {}),
    )
    y = np.concatenate([res.results[i]["out"] for i in range(NCORES)], axis=0)
    if _want_trace:
        return y, res
    return y


# revision 19
# speedup vs baseline: 2.7576x; 1.2115x over previous
"""Trainium2 Bass kernel for nn_CABlock (channel attention / XCA block).

Reference computation (per batch b):
  qkv = x @ qkv_w.T                      # [N, 3C], token-major
  q,k,v per head: [d=64, N] channel-major after reshape/transpose
  q,k l2-normalized over N; attn = softmax((q @ k.T) * temperature, axis=-1)
  out = attn @ v  -> [N, C];  y = out @ proj_w.T + proj_b

Numerics / restructure:
  * l2norm commutes with the bilinear form:
      logits = diag(inv_q) @ (q_raw @ k_raw.T) @ diag(inv_k) * temp
    so any uniform scaling of q/k (from fp8 pre-scaling) cancels exactly.
  * q/k projection and the token Gram run in fp8 (e4m3) DoubleRow mode:
    256-deep contraction at 0.5 PE cycles per output column, 4x bf16
    throughput.  Softmax over the tiny normalized logits washes out the
    quantization noise (validated ~4.5e-3 rel err end to end).  Device
    float8e4 is IEEE e4m3 (max 240, overflow -> inf, no saturation), so
    power-of-two pre-scales keep all fp8 values under ~100.
  * v projection is a 3-term residual-corrected fp8 product
      v = [W8@x8 + W8@xr8 + Wr8@x8] / (SX*SWV)
    with x8=fp8(SX*x), xr8=fp8(SX*x-x8), W8=fp8(SWV*W), Wr8=fp8(SWV*W-W8)
    all host-prepared; the dropped Wr*xr term is second order.
  * per head ONE Gram matmul with stationary=moving=(q_h|k_h) packed along
    the free axis (k|q for odd heads): the cross block is the logits and
    the diagonals of the self blocks are the q/k norms -- free.  The
    parity swap places odd heads' logits on partitions 64-127 so ALL
    later phase-B tensors live on 128 partitions, partition-aligned.
  * attn@v + projection are fused associatively:
      y = v_cm^T @ (blockdiag(probs) @ proj_w^T)
    The tiny per-batch G = BD@pwT costs 4 matmuls of 512 columns instead
    of mixing attention into the N=4096-wide v, eliminating the whole
    attn@v stage and its PSUM-extraction copies.
  * x arrives host-transposed channel-major (DoubleRow k-tile packed); no
    on-device transposes anywhere.  The phase-B scalar chain of batch b
    is interleaved into the following batch's phase A (or the previous
    batch's projection loop for the last batch) so the PE never waits.

Sharding: data-parallel over batch B=16 across 8 cores (2 batches/core).
No collectives needed.
"""

import os
import sys

import numpy as np

for _p in ("/opt/trn_rl_repo", "/root/.axon_site/_ro/trn_rl_repo"):
    if os.path.isdir(_p) and _p not in sys.path:
        sys.path.insert(0, _p)

import concourse.bass as bass  # noqa: E402
from concourse import mybir  # noqa: E402
from concourse.bass import ts  # noqa: E402
from concourse.bass_utils import run_bass_kernel_spmd  # noqa: E402
from concourse.masks import make_identity  # noqa: E402
from concourse.tile import TileContext  # noqa: E402

B, N, C = 16, 4096, 512
H, D = 8, 64
C3 = 3 * C
NCORES = 8
BL = B // NCORES  # batches per core
EPS = 1e-12
NCHUNK = N // 128  # 32 token chunks per batch
NPAIR = NCHUNK // 2
F32 = mybir.dt.float32
BF16 = mybir.dt.bfloat16
F8 = mybir.dt.float8e4
DR = mybir.MatmulPerfMode.DoubleRow

# fp8 pre-scales (powers of two; exactly cancelled on-device). Device
# float8e4 is IEEE e4m3: max finite 240, NO saturation (overflow -> inf),
# so scales keep every fp8 value comfortably under ~100.
SX = 2.0  # x  (|2x| <~ 11)
SWQ = 16.0  # q,k weight columns (|32*q| <~ 80 for the fp8 qks re-quant)
SWV = 256.0  # v weight columns (|256*wv| <~ 26)

MM_MODE = "bf16"  # kept for test.py compatibility


def legalize_waits(nc):
    """Walrus in this environment rejects instructions carrying more than one
    semaphore wait ("Too many sync wait commands"), and rejects sem-ge waits
    on Drain instructions entirely. Tile emits both. Hoist the offending
    waits onto standalone EventSemaphore instructions inserted immediately
    before the instruction on the same engine queue — semantically identical
    (the engine executes the waits, then the instruction)."""
    n_new = 0
    for bb in nc.main_func.blocks:
        il = bb.instructions
        new_list = []
        for ins in il:
            si = ins.sync_info
            waits = list(si.on_wait) if si is not None and si.on_wait else []
            if waits:
                tname = type(ins).__name__
                no_wait_slots = tname in ("InstDrain", "InstDmaTransposeAnt") or (
                    getattr(ins, "opcode", "") in ("Drain", "DmaTransposeAnt")
                )
                keep_budget = 0 if no_wait_slots else 1
                if len(waits) > keep_budget:
                    hoist, keep = waits[:-keep_budget] if keep_budget else waits, (
                        waits[-keep_budget:] if keep_budget else []
                    )
                    for w in hoist:
                        ev = mybir.InstEventSemaphore(
                            name=f"{ins.name}-hoistw{n_new}",
                            ins=[],
                            outs=[],
                            engine=ins.engine,
                            sync_info=mybir.SyncInfo(on_wait=[w], on_update=[]),
                        )
                        new_list.append(ev)
                        n_new += 1
                    ins.sync_info = mybir.SyncInfo(
                        on_wait=keep, on_update=list(si.on_update or [])
                    )
            new_list.append(ins)
        il.clear()
        il.extend(new_list)
    return n_new


def build_bass():
    nc = bass.Bass(trn_type="TRN2")
    # channel-major, DoubleRow k-tile packed: [b, kc2, k, t, n] = channel
    # kc2*256 + t*128 + k of batch b (value pre-scaled by SX, fp8)
    xq8 = nc.dram_tensor("xq8", [BL, 2, 128, 2, N], F8, kind="ExternalInput")
    xr8 = nc.dram_tensor("xr8", [BL, 2, 128, 2, N], F8, kind="ExternalInput")
    # q,k weight cols (x SWQ): [kc2, k, t, j] = qkv_wt[kc2*256+t*128+k, j]
    wqk8 = nc.dram_tensor("wqk8", [2, 128, 2, 2 * C], F8, kind="ExternalInput")
    wv8 = nc.dram_tensor("wv8", [2, 128, 2, C], F8, kind="ExternalInput")
    wvr8 = nc.dram_tensor("wvr8", [2, 128, 2, C], F8, kind="ExternalInput")
    # temperature pre-arranged [par, j]: par 0 = odd heads, par 1 = even
    temp = nc.dram_tensor("temperature", [2, 4], F32, kind="ExternalInput")
    proj_wt = nc.dram_tensor("proj_wt", [C, C], BF16, kind="ExternalInput")
    proj_b = nc.dram_tensor("proj_b", [C], F32, kind="ExternalInput")
    # bf16 output halves the y writeback (the projection loop is otherwise
    # DMA-throttled); the host upcasts to f32
    out = nc.dram_tensor("out", [BL, N, C], BF16, kind="ExternalOutput")

    with TileContext(nc) as tc:
        consts = tc.alloc_tile_pool(name="consts", bufs=1)
        xtp = tc.alloc_tile_pool(name="xtp", bufs=2)
        vtp = tc.alloc_tile_pool(name="vtp", bufs=2)
        qksp = tc.alloc_tile_pool(name="qksp", bufs=4)
        small = tc.alloc_tile_pool(name="small", bufs=2)
        yp = tc.alloc_tile_pool(name="yp", bufs=4)
        ps = tc.alloc_tile_pool(name="ps", bufs=6, space="PSUM")
        accp = tc.alloc_tile_pool(name="accp", bufs=1, space="PSUM")

        # ---- tiny constants (cheap DMAs first) ----
        temp_sb = consts.tile([1, 2, 4], F32)
        nc.sync.dma_start(out=temp_sb, in_=temp[:])
        bias_row = consts.tile([1, C], F32)
        nc.sync.dma_start(out=bias_row, in_=proj_b[:])
        ident_f = consts.tile([128, 128], F32)
        make_identity(nc, ident_f)
        ones_col = consts.tile([128, 1], BF16)
        nc.vector.memset(ones_col, 1.0)
        ones_row = consts.tile([1, 128], BF16)
        nc.vector.memset(ones_row, 1.0)
        ones_f32 = consts.tile([1, 128], F32)
        nc.vector.memset(ones_f32, 1.0)

        # ---- batch-0 x granule j=0 before the bulk weight loads so chunk 0
        # can start as early as possible ----
        def make_x_tiles():
            xq = [xtp.tile([128, 2, N], F8, tag=f"xq{g}", name=f"xq{g}") for g in range(2)]
            xr = [xtp.tile([128, 2, N], F8, tag=f"xr{g}", name=f"xr{g}") for g in range(2)]
            return xq, xr

        def emit_xq_granule(b, xq, j):
            for g in range(2):
                nc.sync.dma_start(out=xq[g][:, :, ts(j, 512)], in_=xq8[b, g, :, :, ts(j, 512)])

        def emit_xr_granule(b, xr, j):
            for g in range(2):
                nc.sync.dma_start(out=xr[g][:, :, ts(j, 512)], in_=xr8[b, g, :, :, ts(j, 512)])

        # chunk 0 only needs xq granule 0 + the q/k weights (v jobs lag by 4
        # chunks), so those two loads go first
        x0 = make_x_tiles()
        emit_xq_granule(0, x0[0], 0)
        wqk = [consts.tile([128, 2, 2 * C], F8, tag=f"wqk{g}", name=f"wqk{g}") for g in range(2)]
        for g in range(2):
            nc.sync.dma_start(out=wqk[g], in_=wqk8[g])
        wv = [consts.tile([128, 2, C], F8, tag=f"wv{g}", name=f"wv{g}") for g in range(2)]
        wvr = [consts.tile([128, 2, C], F8, tag=f"wvr{g}", name=f"wvr{g}") for g in range(2)]
        emit_xr_granule(0, x0[1], 0)
        for g in range(2):
            nc.sync.dma_start(out=wv[g], in_=wv8[g])
            nc.sync.dma_start(out=wvr[g], in_=wvr8[g])

        proj_wT = [consts.tile([128, C], BF16, tag=f"projwT{i}", name=f"projwT{i}") for i in range(4)]
        bias_bc = consts.tile([128, C], F32)

        def emit_deferred_consts():
            # proj weights + bias broadcast: first needed ~100us in
            for kc in range(4):
                nc.sync.dma_start(out=proj_wT[kc], in_=proj_wt[ts(kc, 128), :])
            bias_ps = ps.tile([128, C], F32, tag="ps")
            nc.tensor.matmul(bias_ps, ones_f32, bias_row, start=True, stop=True)
            nc.vector.tensor_copy(out=bias_bc, in_=bias_ps)

        def phase_a(b, interleave, xpre=None):
            """Stream one batch through qkv projection + Gram accumulation.
            ``interleave[i]`` (previous batch's phase-B scalar stages) is
            emitted after chunk i so its ACT/DVE work hides under PE time."""
            if xpre is None:
                xq, xr = make_x_tiles()
                emit_xq_granule(b, xq, 0)
                emit_xr_granule(b, xr, 0)
            else:
                xq, xr = xpre
            for j in range(1, N // 512):
                emit_xq_granule(b, xq, j)
                emit_xr_granule(b, xr, j)
            vT = [vtp.tile([128, N], BF16, tag=f"vt{g}", name=f"vt{g}") for g in range(4)]

            # Gram accumulator: acc2[:, h, :] = s_h^T s_h with s_h=(q_h|k_h)
            # for even h, (k_h|q_h) for odd h.  2 PSUM banks.
            acc2 = accp.tile([128, H, 128], F32, tag="acc")
            qks_tiles = {}

            def emit_gram(p):
                qks = qks_tiles.pop(p)
                for h in range(H):
                    nc.tensor.matmul(
                        acc2[:, h, :],
                        qks[:, :, h],
                        qks[:, :, h],
                        start=(p == 0 and h % 4 == 0),
                        stop=(p == NPAIR - 1 and h % 4 == 3),
                        perf_mode=DR,
                    )

            def qks_pack_aps(qks, t, src):
                """Combined APs for the parity-swapped (q|k)/(k|q) packing:
                one strided copy per source instead of two.  Head h=2j+par
                of q goes to slot s=par; of k to slot s=1-par."""
                base = qks[:, t, 0, 0, :]  # [128, 64] at (t, h=0, s=0)
                hs = qks[:, t, 1, 0, :].offset - base.offset  # h stride
                ss = qks[:, t, 0, 1, :].offset - base.offset  # s stride
                p_ap = list(base.ap[0])
                d_ap = list(base.ap[1])
                q_out = bass.AP(
                    tensor=base.tensor, offset=base.offset,
                    ap=[p_ap, [2 * hs, 4], [hs + ss, 2], d_ap],
                )
                k_out = bass.AP(
                    tensor=base.tensor, offset=base.offset + ss,
                    ap=[p_ap, [2 * hs, 4], [hs - ss, 2], d_ap],
                )
                sb = src[:, 0:64]
                s_ap = [list(sb.ap[0]), [128, 4], [64, 2], list(sb.ap[1])]
                s_view = bass.AP(tensor=sb.tensor, offset=sb.offset, ap=s_ap)
                return q_out, k_out, s_view

            def vjob(cj):
                # v projection: 3-term residual-corrected fp8 DoubleRow
                nj, mc = cj // 4, cj % 4
                vps = ps.tile([128, 512], F32, tag="ps")
                terms = [(wv, xq), (wv, xr), (wvr, xq)]
                for ti, (wt, xt) in enumerate(terms):
                    for g in range(2):
                        nc.tensor.matmul(
                            vps,
                            wt[g][:, :, ts(mc, 128)],
                            xt[g][:, :, ts(nj, 512)],
                            start=(ti == 0 and g == 0),
                            stop=(ti == 2 and g == 1),
                            perf_mode=DR,
                        )
                # descale to true v while extracting; alternate engines to
                # keep both ACT and DVE under the PE chunk rate
                if cj % 2 == 0:
                    nc.scalar.mul(out=vT[mc][:, ts(nj, 512)], in_=vps, mul=1.0 / (SX * SWV))
                else:
                    nc.vector.tensor_scalar_mul(
                        vT[mc][:, ts(nj, 512)], vps, 1.0 / (SX * SWV)
                    )

            for ci in range(NCHUNK):
                t = ci % 2
                # q,k projection (fp8 DoubleRow, stationary = x chunk)
                qp = ps.tile([128, C], F32, tag="ps")
                kp = ps.tile([128, C], F32, tag="ps")
                for g in range(2):
                    nc.tensor.matmul(
                        qp, xq[g][:, :, ts(ci, 128)], wqk[g][:, :, 0:C],
                        start=(g == 0), stop=(g == 1), perf_mode=DR,
                    )
                for g in range(2):
                    nc.tensor.matmul(
                        kp, xq[g][:, :, ts(ci, 128)], wqk[g][:, :, C : 2 * C],
                        start=(g == 0), stop=(g == 1), perf_mode=DR,
                    )
                # fp8 (q_h|k_h) / (k_h|q_h) packing (parity-swapped so odd
                # heads' logits land on partitions 64-127)
                if t == 0:
                    qks_tiles[ci // 2] = qksp.tile(
                        [128, 2, H, 2, 64], F8, tag="qks", name=f"qks{ci // 2}"
                    )
                qks = qks_tiles[ci // 2]
                q_out, k_out, q_in = qks_pack_aps(qks, t, qp)
                _, _, k_in = qks_pack_aps(qks, t, kp)
                nc.scalar.copy(out=q_out, in_=q_in)
                nc.vector.tensor_copy(out=k_out, in_=k_in)

                # v jobs lag 4 chunks so phase A can start before the v
                # weights and x residuals finish loading
                if ci >= 4:
                    vjob(ci - 4)

                # Gram for the previous chunk pair (its copies are done)
                if t == 0 and ci >= 2:
                    emit_gram(ci // 2 - 1)
                if t == 0 and ci // 2 < len(interleave):
                    interleave[ci // 2]()
            for cj in range(NCHUNK - 4, NCHUNK):
                vjob(cj)
            emit_gram(NPAIR - 1)

            # extract logits + masked diagonals immediately so the next batch
            # can reuse the accumulator banks.  Layout notes (j = h//2):
            #   even h: q^T k at [0:64, 64:128], qq at [0:64,0:64], kk at
            #           [64:128,64:128]
            #   odd h:  q^T k (as [dq,e]) at [64:128, 0:64], kk at
            #           [0:64,0:64], qq at [64:128,64:128]
            attn = small.tile([128, 4, 64], F32, tag="attn")
            nc.scalar.copy(out=attn[0:64], in_=acc2[0:64, 0::2, 64:128])
            nc.scalar.copy(out=attn[64:128], in_=acc2[64:128, 1::2, 0:64])
            maskq = small.tile([128, 4, 64], BF16, tag="maskq")
            maskk = small.tile([128, 4, 64], BF16, tag="maskk")
            iq = ident_f[0:64, 0:64]
            iq_bc = bass.AP(
                tensor=iq.tensor, offset=iq.offset,
                ap=[list(iq.ap[0]), [0, 4], list(iq.ap[1])],
            )
            ik = ident_f[64:128, 64:128]
            ik_bc = bass.AP(
                tensor=ik.tensor, offset=ik.offset,
                ap=[list(ik.ap[0]), [0, 4], list(ik.ap[1])],
            )
            mul = mybir.AluOpType.mult
            nc.vector.tensor_tensor(out=maskq[0:64], in0=acc2[0:64, 0::2, 0:64], in1=iq_bc, op=mul)
            nc.vector.tensor_tensor(out=maskq[64:128], in0=acc2[64:128, 1::2, 64:128], in1=ik_bc, op=mul)
            nc.vector.tensor_tensor(out=maskk[0:64], in0=acc2[0:64, 1::2, 0:64], in1=iq_bc, op=mul)
            nc.vector.tensor_tensor(out=maskk[64:128], in0=acc2[64:128, 0::2, 64:128], in1=ik_bc, op=mul)
            return attn, maskq, maskk, vT

        def phase_b_stages(b, attn, maskq, maskk, vT):
            """Returns ([s...] scalar stages to interleave elsewhere, and
            final_block(interleave2) = the fused projection loop)."""
            ssqd = small.tile([128, 4], F32, tag="ssqd")
            invq = small.tile([128, 4], F32, tag="invq")
            # kr[par]: par 0 = odd heads (from maskk top), 1 = even (bottom)
            kr = small.tile([1, 2, 4, 64], F32, tag="kr")
            kr_b = small.tile([1, 2, 4, 64], BF16, tag="krb")
            ikb = small.tile([128, 4, 64], F32, tag="ikb")
            probs = small.tile([128, 4, 64], BF16, tag="probs")
            bd = small.tile([128, 4, 128], BF16, tag="bd")
            gsb = small.tile([128, 4, C], BF16, tag="gsb")
            mx = small.tile([128, 4], F32, tag="mx")
            ex = small.tile([128, 4, 64], F32, tag="ex")
            rs = small.tile([128, 4], F32, tag="rs")

            def s1():
                # ssq_k rows via ones^T @ masked k diags (one per parity);
                # PE ops first so they only depend on the masks
                kr_ps = ps.tile([1, 2, 4, 64], F32, tag="ps")
                nc.tensor.matmul(kr_ps[:, 0], ones_col[0:64, :], maskk[0:64], start=True, stop=True)
                nc.tensor.matmul(kr_ps[:, 1], ones_col[64:128, :], maskk[64:128], start=True, stop=True)
                nc.vector.tensor_copy(out=kr, in_=kr_ps)
                # inv_q = 1/max(sqrt(ssq_q), eps) per (d, h), partition-major
                nc.vector.tensor_reduce(
                    out=ssqd, in_=maskq, axis=mybir.AxisListType.X,
                    op=mybir.AluOpType.add,
                )
                nc.scalar.sqrt(out=invq, in_=ssqd)
                nc.vector.tensor_scalar_max(invq, invq, EPS)
                nc.vector.reciprocal(out=invq, in_=invq)

            def s2():
                # inv_k rows, temperature folded in (uniform over d AND e)
                nc.scalar.sqrt(out=kr, in_=kr)
                nc.vector.tensor_scalar_max(kr, kr, EPS)
                nc.vector.reciprocal(out=kr, in_=kr)
                temp_bc = bass.AP(
                    tensor=temp_sb.tensor, offset=temp_sb.offset,
                    ap=[list(temp_sb.ap[0]), [4, 2], [1, 4], [0, D]],
                )
                nc.vector.tensor_tensor(
                    out=kr_b, in0=kr, in1=temp_bc, op=mybir.AluOpType.mult
                )

            def s3():
                # broadcast inv_k*temp over d-partitions: top half needs even
                # heads (par 1), bottom half odd heads (par 0)
                ikb_ps = ps.tile([128, 4, 64], F32, tag="ps")
                nc.tensor.matmul(
                    ikb_ps[0:64], ones_row[:, 0:64],
                    kr_b[:, 1].rearrange("p h d -> p (h d)"),
                    start=True, stop=True,
                )
                nc.tensor.matmul(
                    ikb_ps[64:128], ones_row[:, 0:64],
                    kr_b[:, 0].rearrange("p h d -> p (h d)"),
                    start=True, stop=True,
                )
                nc.vector.tensor_copy(out=ikb, in_=ikb_ps)
                nc.vector.tensor_mul(out=attn, in0=attn, in1=ikb)
                invq_bc = bass.AP(
                    tensor=invq.tensor, offset=invq.offset,
                    ap=[list(invq.ap[0]), list(invq.ap[1]), [0, D]],
                )
                nc.vector.tensor_tensor(
                    out=attn, in0=attn, in1=invq_bc, op=mybir.AluOpType.mult
                )

            def s4():
                # softmax over the last axis (per head)
                nc.vector.tensor_reduce(
                    out=mx, in_=attn, axis=mybir.AxisListType.X,
                    op=mybir.AluOpType.max, negate=True,
                )
                mx_bc = bass.AP(
                    tensor=mx.tensor, offset=mx.offset,
                    ap=[list(mx.ap[0]), list(mx.ap[1]), [0, D]],
                )
                nc.vector.tensor_tensor(
                    out=attn, in0=attn, in1=mx_bc, op=mybir.AluOpType.add
                )
                nc.scalar.activation(
                    out=ex, in_=attn, func=mybir.ActivationFunctionType.Exp
                )
                nc.vector.tensor_reduce(
                    out=rs, in_=ex, axis=mybir.AxisListType.X,
                    op=mybir.AluOpType.add,
                )
                nc.vector.reciprocal(out=rs, in_=rs)
                rs_bc = bass.AP(
                    tensor=rs.tensor, offset=rs.offset,
                    ap=[list(rs.ap[0]), list(rs.ap[1]), [0, D]],
                )
                nc.vector.tensor_tensor(
                    out=probs, in0=ex, in1=rs_bc, op=mybir.AluOpType.mult
                )

            def s5():
                # blockdiag(probs) [dq, e] per pair: even head at [0:64,0:64],
                # odd head at [64:128,64:128] -- partition-aligned copies
                nc.vector.memset(bd, 0.0)
                nc.scalar.copy(out=bd[0:64, :, 0:64], in_=probs[0:64])
                nc.scalar.copy(out=bd[64:128, :, 64:128], in_=probs[64:128])

            def make_g(gg):
                def sg():
                    # G_g = BD_g @ pwT_g : mix attention into the projection
                    for g in (2 * gg, 2 * gg + 1):
                        g_ps = ps.tile([128, C], F32, tag="ps")
                        nc.tensor.matmul(g_ps, bd[:, g, :], proj_wT[g], start=True, stop=True)
                        if g % 2 == 0:
                            nc.scalar.copy(out=gsb[:, g, :], in_=g_ps)
                        else:
                            nc.vector.tensor_copy(out=gsb[:, g, :], in_=g_ps)
                return sg

            def final_block(interleave2):
                for nj in range(NCHUNK):
                    ypt = ps.tile([128, C], F32, tag="ps")
                    for g in range(4):
                        nc.tensor.matmul(
                            ypt, vT[g][:, ts(nj, 128)], gsb[:, g, :],
                            start=(g == 0), stop=(g == 3),
                        )
                    ysb = yp.tile([128, C], BF16, tag="ysb")
                    nc.vector.tensor_add(out=ysb, in0=ypt, in1=bias_bc)
                    nc.sync.dma_start(out=out[b, ts(nj, 128), :], in_=ysb)
                    if nj % 2 == 0 and nj // 2 < len(interleave2):
                        interleave2[nj // 2]()

            stages = [s1, s2, s3, s4, s5, make_g(0), make_g(1)]
            return stages, final_block

        prev = None
        for b in range(BL):
            st = prev[0] if prev else []
            ctx = phase_a(b, st, xpre=x0 if b == 0 else None)
            if b == 0:
                emit_deferred_consts()
            newp = phase_b_stages(b, *ctx)
            if prev is not None:
                # previous batch's projection loop; on the last batch also
                # hide the current batch's scalar chain inside it
                prev[1](newp[0] if b == BL - 1 else [])
                if b == BL - 1:
                    newp = ([], newp[1])
            prev = newp
        for s in prev[0]:
            s()
        prev[1]([])

        accp.release()
        ps.release()
        yp.release()
        small.release()
        qksp.release()
        vtp.release()
        xtp.release()
        consts.release()

    legalize_waits(nc)
    return nc


def build_trivial_bass():
    """Minimal kernel used by the benchmark harness to measure the
    per-dispatch floor (axon round trip + runtime overhead)."""
    nc = bass.Bass(trn_type="TRN2")
    inp = nc.dram_tensor("inp", [128, 512], F32, kind="ExternalInput")
    outp = nc.dram_tensor("outp", [128, 512], F32, kind="ExternalOutput")
    with TileContext(nc) as tc:
        with tc.tile_pool(name="p", bufs=1) as pool:
            s = pool.tile([128, 512], F32)
            nc.sync.dma_start(out=s, in_=inp[:, :])
            nc.sync.dma_start(out=outp[:, :], in_=s)
    legalize_waits(nc)
    return nc


_NC_CACHE = {}


def _pack_rows(a):
    """[C, cols] -> [kc2, k, t, cols] with row c = kc2*256 + t*128 + k."""
    Crows, cols = a.shape
    return np.ascontiguousarray(
        a.reshape(2, 2, 128, cols).transpose(0, 2, 1, 3)
    )


def make_in_maps(x, qkv_w, temperature, proj_w, proj_b):
    import ml_dtypes

    f8 = ml_dtypes.float8_e4m3  # matches device float8e4 (IEEE e4m3)
    bf = ml_dtypes.bfloat16
    x = np.asarray(x, np.float32)
    qkv_wt = np.asarray(qkv_w, np.float32).T  # [C, 3C]
    tf = np.asarray(temperature, np.float32).reshape(H)
    # [par, j]: par 0 = odd heads, par 1 = even heads
    temp_arr = np.ascontiguousarray(np.stack([tf[1::2], tf[0::2]]))
    proj_wt = np.ascontiguousarray(np.asarray(proj_w, np.float32).T.astype(bf))
    pb = np.ascontiguousarray(np.asarray(proj_b, np.float32))

    wqk8 = _pack_rows(SWQ * qkv_wt[:, 0 : 2 * C]).astype(f8)
    wv_s = SWV * qkv_wt[:, 2 * C :]
    wv8 = wv_s.astype(f8)
    wvr8 = _pack_rows(wv_s - wv8.astype(np.float32)).astype(f8)
    wv8 = _pack_rows(wv8.astype(np.float32)).astype(f8)

    in_maps = []
    for i in range(NCORES):
        xs = SX * x[i * BL : (i + 1) * BL].transpose(0, 2, 1)  # [BL, C, N]
        x8 = xs.astype(f8)
        xr8f = xs - x8.astype(np.float32)
        xq8 = np.stack([_pack_rows(x8[bb].astype(np.float32)) for bb in range(BL)]).astype(f8)
        xr8 = np.stack([_pack_rows(xr8f[bb]) for bb in range(BL)]).astype(f8)
        in_maps.append(
            {
                "xq8": xq8,
                "xr8": xr8,
                "wqk8": wqk8,
                "wv8": wv8,
                "wvr8": wvr8,
                "temperature": temp_arr,
                "proj_wt": proj_wt,
                "proj_b": pb,
            }
        )
    return in_maps


def kernel(x, qkv_w, temperature, proj_w, proj_b, _want_trace=False, _trace_kwargs=None):
    key = MM_MODE
    if key not in _NC_CACHE:
        _NC_CACHE[key] = build_bass()
    nc = _NC_CACHE[key]

    in_maps = make_in_maps(x, qkv_w, temperature, proj_w, proj_b)
    res = run_bass_kernel_spmd(
        nc,
        in_maps,
        core_ids=list(range(NCORES)),
        trace=_want_trace,
        **(_trace_kwargs or {}),
    )
    y = np.concatenate(
        [res.results[i]["out"].astype(np.float32) for i in range(NCORES)], axis=0
    )
    if _want_trace:
        return y, res
    return y


# revision 23
# speedup vs baseline: 2.7755x; 1.0065x over previous
"""Trainium2 Bass kernel for nn_CABlock (channel attention / XCA block).

Reference computation (per batch b):
  qkv = x @ qkv_w.T                      # [N, 3C], token-major
  q,k,v per head: [d=64, N] channel-major after reshape/transpose
  q,k l2-normalized over N; attn = softmax((q @ k.T) * temperature, axis=-1)
  out = attn @ v  -> [N, C];  y = out @ proj_w.T + proj_b

Numerics / restructure:
  * l2norm commutes with the bilinear form:
      logits = diag(inv_q) @ (q_raw @ k_raw.T) @ diag(inv_k) * temp
    so any uniform scaling of q/k (from fp8 pre-scaling) cancels exactly.
  * q/k projection and the token Gram run in fp8 (e4m3) DoubleRow mode:
    256-deep contraction at 0.5 PE cycles per output column, 4x bf16
    throughput.  Softmax over the tiny normalized logits washes out the
    quantization noise (validated ~4.5e-3 rel err end to end).  Device
    float8e4 is IEEE e4m3 (max 240, overflow -> inf, no saturation), so
    power-of-two pre-scales keep all fp8 values under ~100.
  * v projection is a 3-term residual-corrected fp8 product
      v = [W8@x8 + W8@xr8 + Wr8@x8] / (SX*SWV)
    with x8=fp8(SX*x), xr8=fp8(SX*x-x8), W8=fp8(SWV*W), Wr8=fp8(SWV*W-W8)
    all host-prepared; the dropped Wr*xr term is second order.
  * per head ONE Gram matmul with stationary=moving=(q_h|k_h) packed along
    the free axis (k|q for odd heads): the cross block is the logits and
    the diagonals of the self blocks are the q/k norms -- free.  The
    parity swap places odd heads' logits on partitions 64-127 so ALL
    later phase-B tensors live on 128 partitions, partition-aligned.
  * attn@v + projection are fused associatively:
      y = v_cm^T @ (blockdiag(probs) @ proj_w^T)
    The tiny per-batch G = BD@pwT costs 4 matmuls of 512 columns instead
    of mixing attention into the N=4096-wide v, eliminating the whole
    attn@v stage and its PSUM-extraction copies.
  * x arrives host-transposed channel-major (DoubleRow k-tile packed); no
    on-device transposes anywhere.  The phase-B scalar chain of batch b
    is interleaved into the following batch's phase A (or the previous
    batch's projection loop for the last batch) so the PE never waits.

Sharding: data-parallel over batch B=16 across 8 cores (2 batches/core).
No collectives needed.
"""

import os
import sys

import numpy as np

for _p in ("/opt/trn_rl_repo", "/root/.axon_site/_ro/trn_rl_repo"):
    if os.path.isdir(_p) and _p not in sys.path:
        sys.path.insert(0, _p)

import concourse.bass as bass  # noqa: E402
from concourse import mybir  # noqa: E402
from concourse.bass import ts  # noqa: E402
from concourse.bass_utils import run_bass_kernel_spmd  # noqa: E402
from concourse.masks import make_identity  # noqa: E402
from concourse.tile import TileContext  # noqa: E402

B, N, C = 16, 4096, 512
H, D = 8, 64
C3 = 3 * C
NCORES = 8
BL = B // NCORES  # batches per core
EPS = 1e-12
NCHUNK = N // 128  # 32 token chunks per batch
NPAIR = NCHUNK // 2
F32 = mybir.dt.float32
BF16 = mybir.dt.bfloat16
F8 = mybir.dt.float8e4
DR = mybir.MatmulPerfMode.DoubleRow

# fp8 pre-scales (powers of two; exactly cancelled on-device). Device
# float8e4 is IEEE e4m3: max finite 240, NO saturation (overflow -> inf),
# so scales keep every fp8 value comfortably under ~100.
SX = 2.0  # x  (|2x| <~ 11)
SWQ = 16.0  # q,k weight columns (|32*q| <~ 80 for the fp8 qks re-quant)
SWV = 256.0  # v weight columns (|256*wv| <~ 26)

MM_MODE = "bf16"  # kept for test.py compatibility


def legalize_waits(nc):
    """Walrus in this environment rejects instructions carrying more than one
    semaphore wait ("Too many sync wait commands"), and rejects sem-ge waits
    on Drain instructions entirely. Tile emits both. Hoist the offending
    waits onto standalone EventSemaphore instructions inserted immediately
    before the instruction on the same engine queue — semantically identical
    (the engine executes the waits, then the instruction)."""
    n_new = 0
    for bb in nc.main_func.blocks:
        il = bb.instructions
        new_list = []
        for ins in il:
            si = ins.sync_info
            waits = list(si.on_wait) if si is not None and si.on_wait else []
            if waits:
                tname = type(ins).__name__
                no_wait_slots = tname in ("InstDrain", "InstDmaTransposeAnt") or (
                    getattr(ins, "opcode", "") in ("Drain", "DmaTransposeAnt")
                )
                keep_budget = 0 if no_wait_slots else 1
                if len(waits) > keep_budget:
                    hoist, keep = waits[:-keep_budget] if keep_budget else waits, (
                        waits[-keep_budget:] if keep_budget else []
                    )
                    for w in hoist:
                        ev = mybir.InstEventSemaphore(
                            name=f"{ins.name}-hoistw{n_new}",
                            ins=[],
                            outs=[],
                            engine=ins.engine,
                            sync_info=mybir.SyncInfo(on_wait=[w], on_update=[]),
                        )
                        new_list.append(ev)
                        n_new += 1
                    ins.sync_info = mybir.SyncInfo(
                        on_wait=keep, on_update=list(si.on_update or [])
                    )
            new_list.append(ins)
        il.clear()
        il.extend(new_list)
    return n_new


def build_bass():
    nc = bass.Bass(trn_type="TRN2")
    # channel-major, DoubleRow k-tile packed: [b, k, kc2, t, n] = channel
    # kc2*256 + t*128 + k of batch b (value pre-scaled by SX, fp8).  k
    # (the SBUF partition) is outermost so each granule is ONE DMA.
    xq8 = nc.dram_tensor("xq8", [BL, 128, 2, 2, N], F8, kind="ExternalInput")
    xr8 = nc.dram_tensor("xr8", [BL, 128, 2, 2, N], F8, kind="ExternalInput")
    # q,k weight cols (x SWQ): [k, kc2, t, j] = qkv_wt[kc2*256+t*128+k, j]
    wqk8 = nc.dram_tensor("wqk8", [128, 2, 2, 2 * C], F8, kind="ExternalInput")
    wv8 = nc.dram_tensor("wv8", [128, 2, 2, C], F8, kind="ExternalInput")
    wvr8 = nc.dram_tensor("wvr8", [128, 2, 2, C], F8, kind="ExternalInput")
    # temperature pre-arranged [par, j]: par 0 = odd heads, par 1 = even
    temp = nc.dram_tensor("temperature", [2, 4], F32, kind="ExternalInput")
    proj_wt = nc.dram_tensor("proj_wt", [C, C], BF16, kind="ExternalInput")
    proj_b = nc.dram_tensor("proj_b", [C], F32, kind="ExternalInput")
    # bf16 output halves the y writeback (the projection loop is otherwise
    # DMA-throttled); the host upcasts to f32
    out = nc.dram_tensor("out", [BL, N, C], BF16, kind="ExternalOutput")

    with TileContext(nc) as tc:
        consts = tc.alloc_tile_pool(name="consts", bufs=1)
        xtp = tc.alloc_tile_pool(name="xtp", bufs=2)
        vtp = tc.alloc_tile_pool(name="vtp", bufs=2)
        qksp = tc.alloc_tile_pool(name="qksp", bufs=4)
        small = tc.alloc_tile_pool(name="small", bufs=2)
        yp = tc.alloc_tile_pool(name="yp", bufs=4)
        ps = tc.alloc_tile_pool(name="ps", bufs=6, space="PSUM")
        accp = tc.alloc_tile_pool(name="accp", bufs=1, space="PSUM")

        # ---- tiny constants (cheap DMAs first) ----
        temp_sb = consts.tile([1, 2, 4], F32)
        nc.sync.dma_start(out=temp_sb, in_=temp[:])
        bias_row = consts.tile([1, C], F32)
        nc.sync.dma_start(out=bias_row, in_=proj_b[:])
        ident_f = consts.tile([128, 128], F32)
        make_identity(nc, ident_f)
        ones_col = consts.tile([128, 1], BF16)
        nc.vector.memset(ones_col, 1.0)
        ones_row = consts.tile([1, 128], BF16)
        nc.vector.memset(ones_row, 1.0)
        ones_f32 = consts.tile([1, 128], F32)
        nc.vector.memset(ones_f32, 1.0)

        # ---- batch-0 x granule j=0 before the bulk weight loads so chunk 0
        # can start as early as possible ----
        def make_x_tiles():
            # [k, g, t, n]; the q/k matmul lhsT is [:, g, :, chunk]
            xq = xtp.tile([128, 2, 2, N], F8, tag="xq", name="xq")
            xr = xtp.tile([128, 2, 2, N], F8, tag="xr", name="xr")
            return xq, xr

        def emit_xq_granule(b, xq, j):
            nc.sync.dma_start(out=xq[:, :, :, ts(j, 512)], in_=xq8[b, :, :, :, ts(j, 512)])

        def emit_xr_granule(b, xr, j):
            nc.sync.dma_start(out=xr[:, :, :, ts(j, 512)], in_=xr8[b, :, :, :, ts(j, 512)])

        # chunk 0 only needs xq granule 0 + the q/k weights (v jobs lag by 4
        # chunks), so those two loads go first
        x0 = make_x_tiles()
        emit_xq_granule(0, x0[0], 0)
        wqk_t = consts.tile([128, 2, 2, 2 * C], F8, name="wqk_t")
        nc.sync.dma_start(out=wqk_t, in_=wqk8[:])
        wqk = [wqk_t[:, g] for g in range(2)]
        wv_t = consts.tile([128, 2, 2, C], F8, name="wv_t")
        wvr_t = consts.tile([128, 2, 2, C], F8, name="wvr_t")
        emit_xr_granule(0, x0[1], 0)
        nc.sync.dma_start(out=wv_t, in_=wv8[:])
        nc.sync.dma_start(out=wvr_t, in_=wvr8[:])
        wv = [wv_t[:, g] for g in range(2)]
        wvr = [wvr_t[:, g] for g in range(2)]

        proj_wT = [consts.tile([128, C], BF16, tag=f"projwT{i}", name=f"projwT{i}") for i in range(4)]
        bias_bc = consts.tile([128, C], F32)

        def emit_deferred_consts():
            # proj weights + bias broadcast: first needed ~100us in
            for kc in range(4):
                nc.sync.dma_start(out=proj_wT[kc], in_=proj_wt[ts(kc, 128), :])
            bias_ps = ps.tile([128, C], F32, tag="ps")
            nc.tensor.matmul(bias_ps, ones_f32, bias_row, start=True, stop=True)
            nc.vector.tensor_copy(out=bias_bc, in_=bias_ps)

        def phase_a(b, interleave, xpre=None):
            """Stream one batch through qkv projection + Gram accumulation.
            ``interleave[i]`` (previous batch's phase-B scalar stages) is
            emitted after chunk i so its ACT/DVE work hides under PE time."""
            if xpre is None:
                xq, xr = make_x_tiles()
                emit_xq_granule(b, xq, 0)
                emit_xr_granule(b, xr, 0)
            else:
                xq, xr = xpre
            for j in range(1, N // 512):
                emit_xq_granule(b, xq, j)
                emit_xr_granule(b, xr, j)
            vT = [vtp.tile([128, N], BF16, tag=f"vt{g}", name=f"vt{g}") for g in range(4)]

            # Gram accumulator: acc2[:, h, :] = s_h^T s_h with s_h=(q_h|k_h)
            # for even h, (k_h|q_h) for odd h.  2 PSUM banks.
            acc2 = accp.tile([128, H, 128], F32, tag="acc")
            qks_tiles = {}

            def emit_gram(p):
                qks = qks_tiles.pop(p)
                for h in range(H):
                    nc.tensor.matmul(
                        acc2[:, h, :],
                        qks[:, :, h],
                        qks[:, :, h],
                        start=(p == 0 and h % 4 == 0),
                        stop=(p == NPAIR - 1 and h % 4 == 3),
                        perf_mode=DR,
                    )

            def qks_pack_aps(qks, t, src):
                """Combined APs for the parity-swapped (q|k)/(k|q) packing:
                one strided copy per source instead of two.  Head h=2j+par
                of q goes to slot s=par; of k to slot s=1-par."""
                base = qks[:, t, 0, 0, :]  # [128, 64] at (t, h=0, s=0)
                hs = qks[:, t, 1, 0, :].offset - base.offset  # h stride
                ss = qks[:, t, 0, 1, :].offset - base.offset  # s stride
                p_ap = list(base.ap[0])
                d_ap = list(base.ap[1])
                q_out = bass.AP(
                    tensor=base.tensor, offset=base.offset,
                    ap=[p_ap, [2 * hs, 4], [hs + ss, 2], d_ap],
                )
                k_out = bass.AP(
                    tensor=base.tensor, offset=base.offset + ss,
                    ap=[p_ap, [2 * hs, 4], [hs - ss, 2], d_ap],
                )
                sb = src[:, 0:64]
                s_ap = [list(sb.ap[0]), [128, 4], [64, 2], list(sb.ap[1])]
                s_view = bass.AP(tensor=sb.tensor, offset=sb.offset, ap=s_ap)
                return q_out, k_out, s_view

            def vjob(cj):
                # v projection: 3-term residual-corrected fp8 DoubleRow
                nj, mc = cj // 4, cj % 4
                vps = ps.tile([128, 512], F32, tag="ps")
                terms = [(wv, xq), (wv, xr), (wvr, xq)]
                for ti, (wt, xt) in enumerate(terms):
                    for g in range(2):
                        nc.tensor.matmul(
                            vps,
                            wt[g][:, :, ts(mc, 128)],
                            xt[:, g, :, ts(nj, 512)],
                            start=(ti == 0 and g == 0),
                            stop=(ti == 2 and g == 1),
                            perf_mode=DR,
                        )
                # descale to true v while extracting; alternate engines to
                # keep both ACT and DVE under the PE chunk rate
                if cj % 2 == 0:
                    nc.scalar.mul(out=vT[mc][:, ts(nj, 512)], in_=vps, mul=1.0 / (SX * SWV))
                else:
                    nc.vector.tensor_scalar_mul(
                        vT[mc][:, ts(nj, 512)], vps, 1.0 / (SX * SWV)
                    )

            for ci in range(NCHUNK):
                t = ci % 2
                # q,k projection (fp8 DoubleRow, stationary = x chunk)
                qp = ps.tile([128, C], F32, tag="ps")
                kp = ps.tile([128, C], F32, tag="ps")
                for g in range(2):
                    nc.tensor.matmul(
                        qp, xq[:, g, :, ts(ci, 128)], wqk[g][:, :, 0:C],
                        start=(g == 0), stop=(g == 1), perf_mode=DR,
                    )
                for g in range(2):
                    nc.tensor.matmul(
                        kp, xq[:, g, :, ts(ci, 128)], wqk[g][:, :, C : 2 * C],
                        start=(g == 0), stop=(g == 1), perf_mode=DR,
                    )
                # fp8 (q_h|k_h) / (k_h|q_h) packing (parity-swapped so odd
                # heads' logits land on partitions 64-127)
                if t == 0:
                    qks_tiles[ci // 2] = qksp.tile(
                        [128, 2, H, 2, 64], F8, tag="qks", name=f"qks{ci // 2}"
                    )
                qks = qks_tiles[ci // 2]
                q_out, k_out, q_in = qks_pack_aps(qks, t, qp)
                _, _, k_in = qks_pack_aps(qks, t, kp)
                nc.scalar.copy(out=q_out, in_=q_in)
                nc.vector.tensor_copy(out=k_out, in_=k_in)

                # v jobs lag 4 chunks so phase A can start before the v
                # weights and x residuals finish loading
                if ci >= 4:
                    vjob(ci - 4)

                # Gram for the previous chunk pair (its copies are done)
                if t == 0 and ci >= 2:
                    emit_gram(ci // 2 - 1)
                if t == 0 and ci // 2 < len(interleave):
                    interleave[ci // 2]()
            for cj in range(NCHUNK - 4, NCHUNK):
                vjob(cj)
            emit_gram(NPAIR - 1)

            # extract logits + masked diagonals immediately so the next batch
            # can reuse the accumulator banks.  Layout notes (j = h//2):
            #   even h: q^T k at [0:64, 64:128], qq at [0:64,0:64], kk at
            #           [64:128,64:128]
            #   odd h:  q^T k (as [dq,e]) at [64:128, 0:64], kk at
            #           [0:64,0:64], qq at [64:128,64:128]
            attn = small.tile([128, 4, 64], F32, tag="attn")
            nc.scalar.copy(out=attn[0:64], in_=acc2[0:64, 0::2, 64:128])
            nc.scalar.copy(out=attn[64:128], in_=acc2[64:128, 1::2, 0:64])
            maskq = small.tile([128, 4, 64], BF16, tag="maskq")
            maskk = small.tile([128, 4, 64], BF16, tag="maskk")
            iq = ident_f[0:64, 0:64]
            iq_bc = bass.AP(
                tensor=iq.tensor, offset=iq.offset,
                ap=[list(iq.ap[0]), [0, 4], list(iq.ap[1])],
            )
            ik = ident_f[64:128, 64:128]
            ik_bc = bass.AP(
                tensor=ik.tensor, offset=ik.offset,
                ap=[list(ik.ap[0]), [0, 4], list(ik.ap[1])],
            )
            mul = mybir.AluOpType.mult
            nc.vector.tensor_tensor(out=maskq[0:64], in0=acc2[0:64, 0::2, 0:64], in1=iq_bc, op=mul)
            nc.vector.tensor_tensor(out=maskq[64:128], in0=acc2[64:128, 1::2, 64:128], in1=ik_bc, op=mul)
            nc.vector.tensor_tensor(out=maskk[0:64], in0=acc2[0:64, 1::2, 0:64], in1=iq_bc, op=mul)
            nc.vector.tensor_tensor(out=maskk[64:128], in0=acc2[64:128, 0::2, 64:128], in1=ik_bc, op=mul)
            return attn, maskq, maskk, vT

        def phase_b_stages(b, attn, maskq, maskk, vT):
            """Returns ([s...] scalar stages to interleave elsewhere, and
            final_block(interleave2) = the fused projection loop)."""
            ssqd = small.tile([128, 4], F32, tag="ssqd")
            invq = small.tile([128, 4], F32, tag="invq")
            # kr[par]: par 0 = odd heads (from maskk top), 1 = even (bottom)
            kr = small.tile([1, 2, 4, 64], F32, tag="kr")
            kr_b = small.tile([1, 2, 4, 64], BF16, tag="krb")
            ikb = small.tile([128, 4, 64], F32, tag="ikb")
            probs = small.tile([128, 4, 64], BF16, tag="probs")
            bd = small.tile([128, 4, 128], BF16, tag="bd")
            gsb = small.tile([128, 4, C], BF16, tag="gsb")
            mx = small.tile([128, 4], F32, tag="mx")
            ex = small.tile([128, 4, 64], F32, tag="ex")
            rs = small.tile([128, 4], F32, tag="rs")

            def s1():
                # ssq_k rows via ones^T @ masked k diags (one per parity);
                # PE ops first so they only depend on the masks
                kr_ps = ps.tile([1, 2, 4, 64], F32, tag="ps")
                nc.tensor.matmul(kr_ps[:, 0], ones_col[0:64, :], maskk[0:64], start=True, stop=True)
                nc.tensor.matmul(kr_ps[:, 1], ones_col[64:128, :], maskk[64:128], start=True, stop=True)
                nc.vector.tensor_copy(out=kr, in_=kr_ps)
                # inv_q = 1/max(sqrt(ssq_q), eps) per (d, h), partition-major
                nc.vector.tensor_reduce(
                    out=ssqd, in_=maskq, axis=mybir.AxisListType.X,
                    op=mybir.AluOpType.add,
                )
                nc.scalar.sqrt(out=invq, in_=ssqd)
                nc.vector.tensor_scalar_max(invq, invq, EPS)
                nc.vector.reciprocal(out=invq, in_=invq)

            def s2():
                # inv_k rows, temperature folded in (uniform over d AND e)
                nc.scalar.sqrt(out=kr, in_=kr)
                nc.vector.tensor_scalar_max(kr, kr, EPS)
                nc.vector.reciprocal(out=kr, in_=kr)
                temp_bc = bass.AP(
                    tensor=temp_sb.tensor, offset=temp_sb.offset,
                    ap=[list(temp_sb.ap[0]), [4, 2], [1, 4], [0, D]],
                )
                nc.vector.tensor_tensor(
                    out=kr_b, in0=kr, in1=temp_bc, op=mybir.AluOpType.mult
                )

            def s3():
                # broadcast inv_k*temp over d-partitions: top half needs even
                # heads (par 1), bottom half odd heads (par 0)
                ikb_ps = ps.tile([128, 4, 64], F32, tag="ps")
                nc.tensor.matmul(
                    ikb_ps[0:64], ones_row[:, 0:64],
                    kr_b[:, 1].rearrange("p h d -> p (h d)"),
                    start=True, stop=True,
                )
                nc.tensor.matmul(
                    ikb_ps[64:128], ones_row[:, 0:64],
                    kr_b[:, 0].rearrange("p h d -> p (h d)"),
                    start=True, stop=True,
                )
                nc.vector.tensor_copy(out=ikb, in_=ikb_ps)
                nc.vector.tensor_mul(out=attn, in0=attn, in1=ikb)
                invq_bc = bass.AP(
                    tensor=invq.tensor, offset=invq.offset,
                    ap=[list(invq.ap[0]), list(invq.ap[1]), [0, D]],
                )
                nc.vector.tensor_tensor(
                    out=attn, in0=attn, in1=invq_bc, op=mybir.AluOpType.mult
                )

            def s4():
                # softmax over the last axis (per head)
                nc.vector.tensor_reduce(
                    out=mx, in_=attn, axis=mybir.AxisListType.X,
                    op=mybir.AluOpType.max, negate=True,
                )
                mx_bc = bass.AP(
                    tensor=mx.tensor, offset=mx.offset,
                    ap=[list(mx.ap[0]), list(mx.ap[1]), [0, D]],
                )
                nc.vector.tensor_tensor(
                    out=attn, in0=attn, in1=mx_bc, op=mybir.AluOpType.add
                )
                nc.scalar.activation(
                    out=ex, in_=attn, func=mybir.ActivationFunctionType.Exp
                )
                nc.vector.tensor_reduce(
                    out=rs, in_=ex, axis=mybir.AxisListType.X,
                    op=mybir.AluOpType.add,
                )
                nc.vector.reciprocal(out=rs, in_=rs)
                rs_bc = bass.AP(
                    tensor=rs.tensor, offset=rs.offset,
                    ap=[list(rs.ap[0]), list(rs.ap[1]), [0, D]],
                )
                nc.vector.tensor_tensor(
                    out=probs, in0=ex, in1=rs_bc, op=mybir.AluOpType.mult
                )

            def s5():
                # blockdiag(probs) [dq, e] per pair: even head at [0:64,0:64],
                # odd head at [64:128,64:128] -- partition-aligned copies
                nc.vector.memset(bd, 0.0)
                nc.scalar.copy(out=bd[0:64, :, 0:64], in_=probs[0:64])
                nc.scalar.copy(out=bd[64:128, :, 64:128], in_=probs[64:128])

            def make_g(gg):
                def sg():
                    # G_g = BD_g @ pwT_g : mix attention into the projection
                    for g in (2 * gg, 2 * gg + 1):
                        g_ps = ps.tile([128, C], F32, tag="ps")
                        nc.tensor.matmul(g_ps, bd[:, g, :], proj_wT[g], start=True, stop=True)
                        if g % 2 == 0:
                            nc.scalar.copy(out=gsb[:, g, :], in_=g_ps)
                        else:
                            nc.vector.tensor_copy(out=gsb[:, g, :], in_=g_ps)
                return sg

            def final_block(interleave2):
                for nj in range(NCHUNK):
                    ypt = ps.tile([128, C], F32, tag="ps")
                    for g in range(4):
                        nc.tensor.matmul(
                            ypt, vT[g][:, ts(nj, 128)], gsb[:, g, :],
                            start=(g == 0), stop=(g == 3),
                        )
                    ysb = yp.tile([128, C], BF16, tag="ysb")
                    nc.vector.tensor_add(out=ysb, in0=ypt, in1=bias_bc)
                    nc.sync.dma_start(out=out[b, ts(nj, 128), :], in_=ysb)
                    if nj % 2 == 0 and nj // 2 < len(interleave2):
                        interleave2[nj // 2]()

            stages = [s1, s2, s3, s4, s5, make_g(0), make_g(1)]
            return stages, final_block

        prev = None
        for b in range(BL):
            st = prev[0] if prev else []
            ctx = phase_a(b, st, xpre=x0 if b == 0 else None)
            if b == 0:
                emit_deferred_consts()
            newp = phase_b_stages(b, *ctx)
            if prev is not None:
                # previous batch's projection loop; on the last batch also
                # hide the current batch's scalar chain inside it
                prev[1](newp[0] if b == BL - 1 else [])
                if b == BL - 1:
                    newp = ([], newp[1])
            prev = newp
        for s in prev[0]:
            s()
        prev[1]([])

        accp.release()
        ps.release()
        yp.release()
        small.release()
        qksp.release()
        vtp.release()
        xtp.release()
        consts.release()

    legalize_waits(nc)
    return nc


def build_trivial_bass():
    """Minimal kernel used by the benchmark harness to measure the
    per-dispatch floor (axon round trip + runtime overhead)."""
    nc = bass.Bass(trn_type="TRN2")
    inp = nc.dram_tensor("inp", [128, 512], F32, kind="ExternalInput")
    outp = nc.dram_tensor("outp", [128, 512], F32, kind="ExternalOutput")
    with TileContext(nc) as tc:
        with tc.tile_pool(name="p", bufs=1) as pool:
            s = pool.tile([128, 512], F32)
            nc.sync.dma_start(out=s, in_=inp[:, :])
            nc.sync.dma_start(out=outp[:, :], in_=s)
    legalize_waits(nc)
    return nc


_NC_CACHE = {}


def _pack_rows(a):
    """[C, cols] -> [k, kc2, t, cols] with row c = kc2*256 + t*128 + k
    (k outermost so granule loads are single DMAs)."""
    Crows, cols = a.shape
    return np.ascontiguousarray(
        a.reshape(2, 2, 128, cols).transpose(2, 0, 1, 3)
    )


def make_in_maps(x, qkv_w, temperature, proj_w, proj_b):
    import ml_dtypes

    f8 = ml_dtypes.float8_e4m3  # matches device float8e4 (IEEE e4m3)
    bf = ml_dtypes.bfloat16
    x = np.asarray(x, np.float32)
    qkv_wt = np.asarray(qkv_w, np.float32).T  # [C, 3C]
    tf = np.asarray(temperature, np.float32).reshape(H)
    # [par, j]: par 0 = odd heads, par 1 = even heads
    temp_arr = np.ascontiguousarray(np.stack([tf[1::2], tf[0::2]]))
    proj_wt = np.ascontiguousarray(np.asarray(proj_w, np.float32).T.astype(bf))
    pb = np.ascontiguousarray(np.asarray(proj_b, np.float32))

    wqk8 = _pack_rows(SWQ * qkv_wt[:, 0 : 2 * C]).astype(f8)
    wv_s = SWV * qkv_wt[:, 2 * C :]
    wv8 = wv_s.astype(f8)
    wvr8 = _pack_rows(wv_s - wv8.astype(np.float32)).astype(f8)
    wv8 = _pack_rows(wv8.astype(np.float32)).astype(f8)

    in_maps = []
    for i in range(NCORES):
        xs = SX * x[i * BL : (i + 1) * BL].transpose(0, 2, 1)  # [BL, C, N]
        x8 = xs.astype(f8)
        xr8f = xs - x8.astype(np.float32)
        xq8 = np.stack([_pack_rows(x8[bb].astype(np.float32)) for bb in range(BL)]).astype(f8)
        xr8 = np.stack([_pack_rows(xr8f[bb]) for bb in range(BL)]).astype(f8)
        in_maps.append(
            {
                "xq8": xq8,
                "xr8": xr8,
                "wqk8": wqk8,
                "wv8": wv8,
                "wvr8": wvr8,
                "temperature": temp_arr,
                "proj_wt": proj_wt,
                "proj_b": pb,
            }
        )
    return in_maps


def kernel(x, qkv_w, temperature, proj_w, proj_b, _want_trace=False, _trace_kwargs=None):
    key = MM_MODE
    if key not in _NC_CACHE:
        _NC_CACHE[key] = build_bass()
    nc = _NC_CACHE[key]

    in_maps = make_in_maps(x, qkv_w, temperature, proj_w, proj_b)
    res = run_bass_kernel_spmd(
        nc,
        in_maps,
        core_ids=list(range(NCORES)),
        trace=_want_trace,
        **(_trace_kwargs or {}),
    )
    y = np.concatenate(
        [res.results[i]["out"].astype(np.float32) for i in range(NCORES)], axis=0
    )
    if _want_trace:
        return y, res
    return y


# revision 26
# speedup vs baseline: 2.9606x; 1.0667x over previous
"""Trainium2 Bass kernel for nn_CABlock (channel attention / XCA block).

Reference computation (per batch b):
  qkv = x @ qkv_w.T                      # [N, 3C], token-major
  q,k,v per head: [d=64, N] channel-major after reshape/transpose
  q,k l2-normalized over N; attn = softmax((q @ k.T) * temperature, axis=-1)
  out = attn @ v  -> [N, C];  y = out @ proj_w.T + proj_b

Numerics / restructure:
  * l2norm commutes with the bilinear form:
      logits = diag(inv_q) @ (q_raw @ k_raw.T) @ diag(inv_k) * temp
    so any uniform scaling of q/k (from fp8 pre-scaling) cancels exactly.
  * q/k projection and the token Gram run in fp8 (e4m3) DoubleRow mode:
    256-deep contraction at 0.5 PE cycles per output column, 4x bf16
    throughput.  Softmax over the tiny normalized logits washes out the
    quantization noise (validated ~4.5e-3 rel err end to end).  Device
    float8e4 is IEEE e4m3 (max 240, overflow -> inf, no saturation), so
    power-of-two pre-scales keep all fp8 values under ~100.
  * v projection is a 3-term residual-corrected fp8 product
      v = [W8@x8 + W8@xr8 + Wr8@x8] / (SX*SWV)
    with x8=fp8(SX*x), xr8=fp8(SX*x-x8), W8=fp8(SWV*W), Wr8=fp8(SWV*W-W8)
    all host-prepared; the dropped Wr*xr term is second order.
  * per head ONE Gram matmul with stationary=moving=(q_h|k_h) packed along
    the free axis (k|q for odd heads): the cross block is the logits and
    the diagonals of the self blocks are the q/k norms -- free.  The
    parity swap places odd heads' logits on partitions 64-127 so ALL
    later phase-B tensors live on 128 partitions, partition-aligned.
  * attn@v + projection are fused associatively:
      y = v_cm^T @ (blockdiag(probs) @ proj_w^T)
    The tiny per-batch G = BD@pwT costs 4 matmuls of 512 columns instead
    of mixing attention into the N=4096-wide v, eliminating the whole
    attn@v stage and its PSUM-extraction copies.
  * x arrives host-transposed channel-major (DoubleRow k-tile packed); no
    on-device transposes anywhere.  The phase-B scalar chain of batch b
    is interleaved into the following batch's phase A (or the previous
    batch's projection loop for the last batch) so the PE never waits.

Sharding: data-parallel over batch B=16 across 8 cores (2 batches/core).
No collectives needed.
"""

import os
import sys

import numpy as np

for _p in ("/opt/trn_rl_repo", "/root/.axon_site/_ro/trn_rl_repo"):
    if os.path.isdir(_p) and _p not in sys.path:
        sys.path.insert(0, _p)

import concourse.bass as bass  # noqa: E402
from concourse import mybir  # noqa: E402
from concourse.bass import ts  # noqa: E402
from concourse.bass_utils import run_bass_kernel_spmd  # noqa: E402
from concourse.masks import make_identity  # noqa: E402
from concourse.tile import TileContext  # noqa: E402

B, N, C = 16, 4096, 512
H, D = 8, 64
C3 = 3 * C
NCORES = 8
BL = B // NCORES  # batches per core
EPS = 1e-12
NCHUNK = N // 128  # 32 token chunks per batch
NPAIR = NCHUNK // 2
F32 = mybir.dt.float32
BF16 = mybir.dt.bfloat16
F8 = mybir.dt.float8e4
DR = mybir.MatmulPerfMode.DoubleRow

# fp8 pre-scales (powers of two; exactly cancelled on-device). Device
# float8e4 is IEEE e4m3: max finite 240, NO saturation (overflow -> inf),
# so scales keep every fp8 value comfortably under ~100.
SX = 2.0  # x  (|2x| <~ 11)
SWQ = 16.0  # q,k weight columns (|32*q| <~ 80 for the fp8 qks re-quant)
SWV = 256.0  # v weight columns (|256*wv| <~ 26)

MM_MODE = "bf16"  # kept for test.py compatibility


def legalize_waits(nc):
    """Walrus in this environment rejects instructions carrying more than one
    semaphore wait ("Too many sync wait commands"), and rejects sem-ge waits
    on Drain instructions entirely. Tile emits both. Hoist the offending
    waits onto standalone EventSemaphore instructions inserted immediately
    before the instruction on the same engine queue — semantically identical
    (the engine executes the waits, then the instruction)."""
    n_new = 0
    for bb in nc.main_func.blocks:
        il = bb.instructions
        new_list = []
        for ins in il:
            si = ins.sync_info
            waits = list(si.on_wait) if si is not None and si.on_wait else []
            if waits:
                tname = type(ins).__name__
                no_wait_slots = tname in ("InstDrain", "InstDmaTransposeAnt") or (
                    getattr(ins, "opcode", "") in ("Drain", "DmaTransposeAnt")
                )
                keep_budget = 0 if no_wait_slots else 1
                if len(waits) > keep_budget:
                    hoist, keep = waits[:-keep_budget] if keep_budget else waits, (
                        waits[-keep_budget:] if keep_budget else []
                    )
                    for w in hoist:
                        ev = mybir.InstEventSemaphore(
                            name=f"{ins.name}-hoistw{n_new}",
                            ins=[],
                            outs=[],
                            engine=ins.engine,
                            sync_info=mybir.SyncInfo(on_wait=[w], on_update=[]),
                        )
                        new_list.append(ev)
                        n_new += 1
                    ins.sync_info = mybir.SyncInfo(
                        on_wait=keep, on_update=list(si.on_update or [])
                    )
            new_list.append(ins)
        il.clear()
        il.extend(new_list)
    return n_new


def build_bass():
    nc = bass.Bass(trn_type="TRN2")
    # channel-major, DoubleRow k-tile packed: [b, k, kc2, t, n] = channel
    # kc2*256 + t*128 + k of batch b (value pre-scaled by SX, fp8).  k
    # (the SBUF partition) is outermost so each granule is ONE DMA.
    xq8 = nc.dram_tensor("xq8", [BL, 128, 2, 2, N], F8, kind="ExternalInput")
    xr8 = nc.dram_tensor("xr8", [BL, 128, 2, 2, N], F8, kind="ExternalInput")
    # q,k weight cols (x SWQ): [k, kc2, t, j] = qkv_wt[kc2*256+t*128+k, j]
    wqk8 = nc.dram_tensor("wqk8", [128, 2, 2, 2 * C], F8, kind="ExternalInput")
    wv8 = nc.dram_tensor("wv8", [128, 2, 2, C], F8, kind="ExternalInput")
    wvr8 = nc.dram_tensor("wvr8", [128, 2, 2, C], F8, kind="ExternalInput")
    # temperature pre-arranged [par, j]: par 0 = odd heads, par 1 = even
    temp = nc.dram_tensor("temperature", [2, 4], F32, kind="ExternalInput")
    proj_wt = nc.dram_tensor("proj_wt", [C, C], BF16, kind="ExternalInput")
    proj_b = nc.dram_tensor("proj_b", [C], F32, kind="ExternalInput")
    # bf16 output halves the y writeback (the projection loop is otherwise
    # DMA-throttled); the host upcasts to f32
    out = nc.dram_tensor("out", [BL, N, C], BF16, kind="ExternalOutput")

    with TileContext(nc) as tc:
        consts = tc.alloc_tile_pool(name="consts", bufs=1)
        xtp = tc.alloc_tile_pool(name="xtp", bufs=2)
        vtp = tc.alloc_tile_pool(name="vtp", bufs=2)
        qksp = tc.alloc_tile_pool(name="qksp", bufs=4)
        small = tc.alloc_tile_pool(name="small", bufs=2)
        yp = tc.alloc_tile_pool(name="yp", bufs=4)
        ps = tc.alloc_tile_pool(name="ps", bufs=6, space="PSUM")
        accp = tc.alloc_tile_pool(name="accp", bufs=1, space="PSUM")

        # ---- tiny constants (cheap DMAs first) ----
        temp_sb = consts.tile([1, 2, 4], F32)
        nc.sync.dma_start(out=temp_sb, in_=temp[:])
        bias_row = consts.tile([1, C], F32)
        nc.sync.dma_start(out=bias_row, in_=proj_b[:])
        ident_f = consts.tile([128, 128], F32)
        make_identity(nc, ident_f)
        ones_col = consts.tile([128, 1], BF16)
        nc.vector.memset(ones_col, 1.0)
        ones_row = consts.tile([1, 128], BF16)
        nc.vector.memset(ones_row, 1.0)
        ones_f32 = consts.tile([1, 128], F32)
        nc.vector.memset(ones_f32, 1.0)

        # ---- batch-0 x granule j=0 before the bulk weight loads so chunk 0
        # can start as early as possible ----
        def make_x_tiles():
            # [k, g, t, n]; the q/k matmul lhsT is [:, g, :, chunk]
            xq = xtp.tile([128, 2, 2, N], F8, tag="xq", name="xq")
            xr = xtp.tile([128, 2, 2, N], F8, tag="xr", name="xr")
            return xq, xr

        def emit_xq_granule(b, xq, j):
            nc.sync.dma_start(out=xq[:, :, :, ts(j, 512)], in_=xq8[b, :, :, :, ts(j, 512)])

        def emit_xr_granule(b, xr, j):
            nc.sync.dma_start(out=xr[:, :, :, ts(j, 512)], in_=xr8[b, :, :, :, ts(j, 512)])

        # chunk 0 only needs xq granule 0 + the q/k weights (v jobs lag by 4
        # chunks), so those two loads go first
        x0 = make_x_tiles()
        emit_xq_granule(0, x0[0], 0)
        wqk_t = consts.tile([128, 2, 2, 2 * C], F8, name="wqk_t")
        nc.sync.dma_start(out=wqk_t[:, :, :, 0:C], in_=wqk8[:, :, :, 0:C])
        nc.sync.dma_start(out=wqk_t[:, :, :, C : 2 * C], in_=wqk8[:, :, :, C : 2 * C])
        wqk = [wqk_t[:, g] for g in range(2)]
        wv_t = consts.tile([128, 2, 2, C], F8, name="wv_t")
        wvr_t = consts.tile([128, 2, 2, C], F8, name="wvr_t")
        emit_xr_granule(0, x0[1], 0)
        nc.sync.dma_start(out=wv_t, in_=wv8[:])
        nc.sync.dma_start(out=wvr_t, in_=wvr8[:])
        wv = [wv_t[:, g] for g in range(2)]
        wvr = [wvr_t[:, g] for g in range(2)]

        proj_wT = [consts.tile([128, C], BF16, tag=f"projwT{i}", name=f"projwT{i}") for i in range(4)]
        bias_bc = consts.tile([128, C], F32)

        def emit_deferred_consts():
            # proj weights + bias broadcast: first needed ~100us in
            for kc in range(4):
                nc.sync.dma_start(out=proj_wT[kc], in_=proj_wt[ts(kc, 128), :])
            bias_ps = ps.tile([128, C], F32, tag="ps")
            nc.tensor.matmul(bias_ps, ones_f32, bias_row, start=True, stop=True)
            nc.vector.tensor_copy(out=bias_bc, in_=bias_ps)

        def phase_a(b, interleave, xpre=None):
            """Stream one batch through qkv projection + Gram accumulation.
            ``interleave[i]`` (previous batch's phase-B scalar stages) is
            emitted after chunk i so its ACT/DVE work hides under PE time."""
            if xpre is None:
                xq, xr = make_x_tiles()
                emit_xq_granule(b, xq, 0)
                emit_xr_granule(b, xr, 0)
            else:
                xq, xr = xpre
            for j in range(1, N // 512):
                emit_xq_granule(b, xq, j)
                emit_xr_granule(b, xr, j)
            vT = [vtp.tile([128, N], BF16, tag=f"vt{g}", name=f"vt{g}") for g in range(4)]

            # Gram accumulator: acc2[:, h, :] = s_h^T s_h with s_h=(q_h|k_h)
            # for even h, (k_h|q_h) for odd h.  2 PSUM banks.
            acc2 = accp.tile([128, H, 128], F32, tag="acc")
            qks_tiles = {}

            def emit_gram(p):
                qks = qks_tiles.pop(p)
                for h in range(H):
                    nc.tensor.matmul(
                        acc2[:, h, :],
                        qks[:, :, h],
                        qks[:, :, h],
                        start=(p == 0 and h % 4 == 0),
                        stop=(p == NPAIR - 1 and h % 4 == 3),
                        perf_mode=DR,
                    )

            def qks_pack_aps(qks, t, src):
                """Combined APs for the parity-swapped (q|k)/(k|q) packing:
                one strided copy per source instead of two.  Head h=2j+par
                of q goes to slot s=par; of k to slot s=1-par."""
                base = qks[:, t, 0, 0, :]  # [128, 64] at (t, h=0, s=0)
                hs = qks[:, t, 1, 0, :].offset - base.offset  # h stride
                ss = qks[:, t, 0, 1, :].offset - base.offset  # s stride
                p_ap = list(base.ap[0])
                d_ap = list(base.ap[1])
                q_out = bass.AP(
                    tensor=base.tensor, offset=base.offset,
                    ap=[p_ap, [2 * hs, 4], [hs + ss, 2], d_ap],
                )
                k_out = bass.AP(
                    tensor=base.tensor, offset=base.offset + ss,
                    ap=[p_ap, [2 * hs, 4], [hs - ss, 2], d_ap],
                )
                sb = src[:, 0:64]
                s_ap = [list(sb.ap[0]), [128, 4], [64, 2], list(sb.ap[1])]
                s_view = bass.AP(tensor=sb.tensor, offset=sb.offset, ap=s_ap)
                return q_out, k_out, s_view

            def vjob(cj):
                # v projection: 3-term residual-corrected fp8 DoubleRow
                nj, mc = cj // 4, cj % 4
                vps = ps.tile([128, 512], F32, tag="ps")
                terms = [(wv, xq), (wv, xr), (wvr, xq)]
                for ti, (wt, xt) in enumerate(terms):
                    for g in range(2):
                        nc.tensor.matmul(
                            vps,
                            wt[g][:, :, ts(mc, 128)],
                            xt[:, g, :, ts(nj, 512)],
                            start=(ti == 0 and g == 0),
                            stop=(ti == 2 and g == 1),
                            perf_mode=DR,
                        )
                # descale to true v while extracting; alternate engines to
                # keep both ACT and DVE under the PE chunk rate
                if cj % 2 == 0:
                    nc.scalar.mul(out=vT[mc][:, ts(nj, 512)], in_=vps, mul=1.0 / (SX * SWV))
                else:
                    nc.vector.tensor_scalar_mul(
                        vT[mc][:, ts(nj, 512)], vps, 1.0 / (SX * SWV)
                    )

            for ci in range(NCHUNK):
                t = ci % 2
                # q,k projection (fp8 DoubleRow, stationary = x chunk)
                qp = ps.tile([128, C], F32, tag="ps")
                kp = ps.tile([128, C], F32, tag="ps")
                for g in range(2):
                    nc.tensor.matmul(
                        qp, xq[:, g, :, ts(ci, 128)], wqk[g][:, :, 0:C],
                        start=(g == 0), stop=(g == 1), perf_mode=DR,
                    )
                for g in range(2):
                    nc.tensor.matmul(
                        kp, xq[:, g, :, ts(ci, 128)], wqk[g][:, :, C : 2 * C],
                        start=(g == 0), stop=(g == 1), perf_mode=DR,
                    )
                # fp8 (q_h|k_h) / (k_h|q_h) packing (parity-swapped so odd
                # heads' logits land on partitions 64-127)
                if t == 0:
                    qks_tiles[ci // 2] = qksp.tile(
                        [128, 2, H, 2, 64], F8, tag="qks", name=f"qks{ci // 2}"
                    )
                qks = qks_tiles[ci // 2]
                q_out, k_out, q_in = qks_pack_aps(qks, t, qp)
                _, _, k_in = qks_pack_aps(qks, t, kp)
                nc.scalar.copy(out=q_out, in_=q_in)
                nc.vector.tensor_copy(out=k_out, in_=k_in)

                # v jobs lag 4 chunks so phase A can start before the v
                # weights and x residuals finish loading
                if ci >= 4:
                    vjob(ci - 4)

                # Gram for the previous chunk pair (its copies are done)
                if t == 0 and ci >= 2:
                    emit_gram(ci // 2 - 1)
                if ci >= 4 and (ci - 4) % 4 == 0 and (ci - 4) // 4 < len(interleave):
                    interleave[(ci - 4) // 4]()
            for cj in range(NCHUNK - 4, NCHUNK):
                vjob(cj)
            emit_gram(NPAIR - 1)

            # extract logits + masked diagonals immediately so the next batch
            # can reuse the accumulator banks.  Layout notes (j = h//2):
            #   even h: q^T k at [0:64, 64:128], qq at [0:64,0:64], kk at
            #           [64:128,64:128]
            #   odd h:  q^T k (as [dq,e]) at [64:128, 0:64], kk at
            #           [0:64,0:64], qq at [64:128,64:128]
            attn = small.tile([128, 4, 64], F32, tag="attn")
            nc.scalar.copy(out=attn[0:64], in_=acc2[0:64, 0::2, 64:128])
            nc.scalar.copy(out=attn[64:128], in_=acc2[64:128, 1::2, 0:64])
            maskq = small.tile([128, 4, 64], BF16, tag="maskq")
            maskk = small.tile([128, 4, 64], BF16, tag="maskk")
            iq = ident_f[0:64, 0:64]
            iq_bc = bass.AP(
                tensor=iq.tensor, offset=iq.offset,
                ap=[list(iq.ap[0]), [0, 4], list(iq.ap[1])],
            )
            ik = ident_f[64:128, 64:128]
            ik_bc = bass.AP(
                tensor=ik.tensor, offset=ik.offset,
                ap=[list(ik.ap[0]), [0, 4], list(ik.ap[1])],
            )
            mul = mybir.AluOpType.mult
            nc.vector.tensor_tensor(out=maskq[0:64], in0=acc2[0:64, 0::2, 0:64], in1=iq_bc, op=mul)
            nc.vector.tensor_tensor(out=maskq[64:128], in0=acc2[64:128, 1::2, 64:128], in1=ik_bc, op=mul)
            nc.vector.tensor_tensor(out=maskk[0:64], in0=acc2[0:64, 1::2, 0:64], in1=iq_bc, op=mul)
            nc.vector.tensor_tensor(out=maskk[64:128], in0=acc2[64:128, 0::2, 64:128], in1=ik_bc, op=mul)
            return attn, maskq, maskk, vT

        def phase_b_stages(b, attn, maskq, maskk, vT):
            """Returns ([s...] scalar stages to interleave elsewhere, and
            final_block(interleave2) = the fused projection loop)."""
            ssqd = small.tile([128, 4], F32, tag="ssqd")
            invq = small.tile([128, 4], F32, tag="invq")
            # kr[par]: par 0 = odd heads (from maskk top), 1 = even (bottom)
            kr = small.tile([1, 2, 4, 64], F32, tag="kr")
            kr_b = small.tile([1, 2, 4, 64], BF16, tag="krb")
            ikb = small.tile([128, 4, 64], F32, tag="ikb")
            probs = small.tile([128, 4, 64], BF16, tag="probs")
            bd = small.tile([128, 4, 128], BF16, tag="bd")
            gsb = small.tile([128, 4, C], BF16, tag="gsb")
            mx = small.tile([128, 4], F32, tag="mx")
            ex = small.tile([128, 4, 64], F32, tag="ex")
            rs = small.tile([128, 4], F32, tag="rs")

            def s1():
                # ssq_k rows via ones^T @ masked k diags (one per parity);
                # PE ops first so they only depend on the masks
                kr_ps = ps.tile([1, 2, 4, 64], F32, tag="ps")
                nc.tensor.matmul(kr_ps[:, 0], ones_col[0:64, :], maskk[0:64], start=True, stop=True)
                nc.tensor.matmul(kr_ps[:, 1], ones_col[64:128, :], maskk[64:128], start=True, stop=True)
                nc.vector.tensor_copy(out=kr, in_=kr_ps)
                # inv_q = 1/max(sqrt(ssq_q), eps) per (d, h), partition-major
                nc.vector.tensor_reduce(
                    out=ssqd, in_=maskq, axis=mybir.AxisListType.X,
                    op=mybir.AluOpType.add,
                )
                nc.scalar.sqrt(out=invq, in_=ssqd)
                nc.vector.tensor_scalar_max(invq, invq, EPS)
                nc.vector.reciprocal(out=invq, in_=invq)

            def s2():
                # inv_k rows, temperature folded in (uniform over d AND e)
                nc.scalar.sqrt(out=kr, in_=kr)
                nc.vector.tensor_scalar_max(kr, kr, EPS)
                nc.vector.reciprocal(out=kr, in_=kr)
                temp_bc = bass.AP(
                    tensor=temp_sb.tensor, offset=temp_sb.offset,
                    ap=[list(temp_sb.ap[0]), [4, 2], [1, 4], [0, D]],
                )
                nc.vector.tensor_tensor(
                    out=kr_b, in0=kr, in1=temp_bc, op=mybir.AluOpType.mult
                )

            def s3():
                # broadcast inv_k*temp over d-partitions: top half needs even
                # heads (par 1), bottom half odd heads (par 0)
                ikb_ps = ps.tile([128, 4, 64], F32, tag="ps")
                nc.tensor.matmul(
                    ikb_ps[0:64], ones_row[:, 0:64],
                    kr_b[:, 1].rearrange("p h d -> p (h d)"),
                    start=True, stop=True,
                )
                nc.tensor.matmul(
                    ikb_ps[64:128], ones_row[:, 0:64],
                    kr_b[:, 0].rearrange("p h d -> p (h d)"),
                    start=True, stop=True,
                )
                nc.vector.tensor_copy(out=ikb, in_=ikb_ps)
                nc.vector.tensor_mul(out=attn, in0=attn, in1=ikb)
                invq_bc = bass.AP(
                    tensor=invq.tensor, offset=invq.offset,
                    ap=[list(invq.ap[0]), list(invq.ap[1]), [0, D]],
                )
                nc.vector.tensor_tensor(
                    out=attn, in0=attn, in1=invq_bc, op=mybir.AluOpType.mult
                )

            def s4():
                # softmax over the last axis (per head)
                nc.vector.tensor_reduce(
                    out=mx, in_=attn, axis=mybir.AxisListType.X,
                    op=mybir.AluOpType.max, negate=True,
                )
                mx_bc = bass.AP(
                    tensor=mx.tensor, offset=mx.offset,
                    ap=[list(mx.ap[0]), list(mx.ap[1]), [0, D]],
                )
                nc.vector.tensor_tensor(
                    out=attn, in0=attn, in1=mx_bc, op=mybir.AluOpType.add
                )
                nc.scalar.activation(
                    out=ex, in_=attn, func=mybir.ActivationFunctionType.Exp
                )
                nc.vector.tensor_reduce(
                    out=rs, in_=ex, axis=mybir.AxisListType.X,
                    op=mybir.AluOpType.add,
                )
                nc.vector.reciprocal(out=rs, in_=rs)
                rs_bc = bass.AP(
                    tensor=rs.tensor, offset=rs.offset,
                    ap=[list(rs.ap[0]), list(rs.ap[1]), [0, D]],
                )
                nc.vector.tensor_tensor(
                    out=probs, in0=ex, in1=rs_bc, op=mybir.AluOpType.mult
                )

            def s5():
                # blockdiag(probs) [dq, e] per pair: even head at [0:64,0:64],
                # odd head at [64:128,64:128] -- partition-aligned copies
                nc.vector.memset(bd, 0.0)
                nc.scalar.copy(out=bd[0:64, :, 0:64], in_=probs[0:64])
                nc.scalar.copy(out=bd[64:128, :, 64:128], in_=probs[64:128])

            def make_g(gg):
                def sg():
                    # G_g = BD_g @ pwT_g : mix attention into the projection
                    for g in (2 * gg, 2 * gg + 1):
                        g_ps = ps.tile([128, C], F32, tag="ps")
                        nc.tensor.matmul(g_ps, bd[:, g, :], proj_wT[g], start=True, stop=True)
                        if g % 2 == 0:
                            nc.scalar.copy(out=gsb[:, g, :], in_=g_ps)
                        else:
                            nc.vector.tensor_copy(out=gsb[:, g, :], in_=g_ps)
                return sg

            def final_block(interleave2):
                for nj in range(NCHUNK):
                    ypt = ps.tile([128, C], F32, tag="ps")
                    for g in range(4):
                        nc.tensor.matmul(
                            ypt, vT[g][:, ts(nj, 128)], gsb[:, g, :],
                            start=(g == 0), stop=(g == 3),
                        )
                    ysb = yp.tile([128, C], BF16, tag="ysb")
                    nc.vector.tensor_add(out=ysb, in0=ypt, in1=bias_bc)
                    nc.sync.dma_start(out=out[b, ts(nj, 128), :], in_=ysb)
                    if nj >= 2 and (nj - 2) % 4 == 0 and (nj - 2) // 4 < len(interleave2):
                        interleave2[(nj - 2) // 4]()

            stages = [s1, s2, s3, s4, s5, make_g(0), make_g(1)]
            return stages, final_block

        prev = None
        for b in range(BL):
            st = prev[0] if prev else []
            ctx = phase_a(b, st, xpre=x0 if b == 0 else None)
            if b == 0:
                emit_deferred_consts()
            newp = phase_b_stages(b, *ctx)
            if prev is not None:
                # previous batch's projection loop; on the last batch also
                # hide the current batch's scalar chain inside it
                prev[1](newp[0] if b == BL - 1 else [])
                if b == BL - 1:
                    newp = ([], newp[1])
            prev = newp
        for s in prev[0]:
            s()
        prev[1]([])

        accp.release()
        ps.release()
        yp.release()
        small.release()
        qksp.release()
        vtp.release()
        xtp.release()
        consts.release()

    legalize_waits(nc)
    return nc


def build_trivial_bass():
    """Minimal kernel used by the benchmark harness to measure the
    per-dispatch floor (axon round trip + runtime overhead)."""
    nc = bass.Bass(trn_type="TRN2")
    inp = nc.dram_tensor("inp", [128, 512], F32, kind="ExternalInput")
    outp = nc.dram_tensor("outp", [128, 512], F32, kind="ExternalOutput")
    with TileContext(nc) as tc:
        with tc.tile_pool(name="p", bufs=1) as pool:
            s = pool.tile([128, 512], F32)
            nc.sync.dma_start(out=s, in_=inp[:, :])
            nc.sync.dma_start(out=outp[:, :], in_=s)
    legalize_waits(nc)
    return nc


_NC_CACHE = {}


def _pack_rows(a):
    """[C, cols] -> [k, kc2, t, cols] with row c = kc2*256 + t*128 + k
    (k outermost so granule loads are single DMAs)."""
    Crows, cols = a.shape
    return np.ascontiguousarray(
        a.reshape(2, 2, 128, cols).transpose(2, 0, 1, 3)
    )


def make_in_maps(x, qkv_w, temperature, proj_w, proj_b):
    import ml_dtypes

    f8 = ml_dtypes.float8_e4m3  # matches device float8e4 (IEEE e4m3)
    bf = ml_dtypes.bfloat16
    x = np.asarray(x, np.float32)
    qkv_wt = np.asarray(qkv_w, np.float32).T  # [C, 3C]
    tf = np.asarray(temperature, np.float32).reshape(H)
    # [par, j]: par 0 = odd heads, par 1 = even heads
    temp_arr = np.ascontiguousarray(np.stack([tf[1::2], tf[0::2]]))
    proj_wt = np.ascontiguousarray(np.asarray(proj_w, np.float32).T.astype(bf))
    pb = np.ascontiguousarray(np.asarray(proj_b, np.float32))

    wqk8 = _pack_rows(SWQ * qkv_wt[:, 0 : 2 * C]).astype(f8)
    wv_s = SWV * qkv_wt[:, 2 * C :]
    wv8 = wv_s.astype(f8)
    wvr8 = _pack_rows(wv_s - wv8.astype(np.float32)).astype(f8)
    wv8 = _pack_rows(wv8.astype(np.float32)).astype(f8)

    in_maps = []
    for i in range(NCORES):
        xs = SX * x[i * BL : (i + 1) * BL].transpose(0, 2, 1)  # [BL, C, N]
        x8 = xs.astype(f8)
        xr8f = xs - x8.astype(np.float32)
        xq8 = np.stack([_pack_rows(x8[bb].astype(np.float32)) for bb in range(BL)]).astype(f8)
        xr8 = np.stack([_pack_rows(xr8f[bb]) for bb in range(BL)]).astype(f8)
        in_maps.append(
            {
                "xq8": xq8,
                "xr8": xr8,
                "wqk8": wqk8,
                "wv8": wv8,
                "wvr8": wvr8,
                "temperature": temp_arr,
                "proj_wt": proj_wt,
                "proj_b": pb,
            }
        )
    return in_maps


def kernel(x, qkv_w, temperature, proj_w, proj_b, _want_trace=False, _trace_kwargs=None):
    key = MM_MODE
    if key not in _NC_CACHE:
        _NC_CACHE[key] = build_bass()
    nc = _NC_CACHE[key]

    in_maps = make_in_maps(x, qkv_w, temperature, proj_w, proj_b)
    res = run_bass_kernel_spmd(
        nc,
        in_maps,
        core_ids=list(range(NCORES)),
        trace=_want_trace,
        **(_trace_kwargs or {}),
    )
    y = np.concatenate(
        [res.results[i]["out"].astype(np.float32) for i in range(NCORES)], axis=0
    )
    if _want_trace:
        return y, res
    return y
